# revision 1
# baseline (speedup 1.0000x reference)
"""Trainium2 Bass kernel for nn_Attention_29326036697657 (sparse_attention).

Dual-input attention with SE (channel) / SA (spatial) gates.
Sharding: data-parallel over batch B=64 across 8 cores (8 batches/core).

Algebraic folds (vs reference):
  - qxo/qyo/attnx dead -> Wqkv comp 0 unused; vy = vx (reference quirk).
  - dots(q,k)+dots(q2,k) = dots(q*(1+g), k) for both SE and SA gates.
  - softmax without max-subtraction (logits O(1)); denominator via a
    ones column appended to each V head block.
  - bias + 1/scale fixups applied on HOST after gather (bp, bp2=bp@Wp+bp).
  - SA 5x5 conv lowered to two host-built Toeplitz matmuls (TM/768, TX).

Numerics: all-fp16 GEMMs/activations, fp32 psum (fp8 measured >2e-2
absmax-rel per quantized tensor and was rejected). Outputs staged fp16,
upcast + bias on host.

Layout: inputs arrive HOST-pre-transposed ([768, NT] fp16) so the kernel
does zero input transposes. q/k tiles [128, NT] f16; v natural per-batch
[72, 2*780] f16 with denominator columns; z transposed back via PE (f16
identity); proj reads zT slabs, psum evicted f16 and DMA'd per t-chunk.

Schedule: software-pipelined around the ACT exp chain (the non-PE floor):
attention S+exp blocks drip a cost-budgeted weave of V/qk-GEMMs, the SE
chain, and projections (2-block lag) into exp-wait gaps; x-side blocks
interleave into the y-side ACT tail. kx/zTx alias yt16/xt16 SBUF. GPSIMD
cannot touch PSUM, so evictions rotate across DVE/ACT only (DVE-only in
exp-saturated regions).
"""

import sys

sys.path.insert(0, "/opt/trn_rl_repo")

import numpy as np
import ml_dtypes

import concourse.bass as bass
import concourse.bacc as bacc_mod
import concourse.mybir as mybir
import concourse.tile as tile
from concourse.masks import make_identity

# ---------------------------------------------------------------- constants
DIM = 768
HEADS = 12
PATCH = 12
N = PATCH * PATCH          # 144
B = 64
RED = 16
HID = DIM // RED           # 48
HD = DIM // HEADS          # 64
SCALE = HD ** -0.5         # 0.125

NCORES = 8
BC = B // NCORES           # 8 batches per core
NT = BC * N                # 1152 rows per core
CH = DIM // 128            # 6 channel chunks
NF = 384                   # qkv/proj moving chunk
MC = 72                    # m/n half within one batch
ESC = SCALE                # exp scale

F32 = mybir.dt.float32
F16 = mybir.dt.float16
BF16 = mybir.dt.bfloat16
F8 = mybir.dt.float8e4
AX = mybir.AxisListType
AF = mybir.ActivationFunctionType
ALU = mybir.AluOpType
DR = mybir.MatmulPerfMode.DoubleRow

_COMPILED = {}


def build_program():
    nc = bacc_mod.Bacc()

    xT_d = nc.dram_tensor("xT", [DIM, NT], F16, kind="ExternalInput")
    yT_d = nc.dram_tensor("yT", [DIM, NT], F16, kind="ExternalInput")
    wq_d = nc.dram_tensor("wq", [DIM, DIM], F16, kind="ExternalInput")
    wk_d = nc.dram_tensor("wk", [DIM, DIM], F16, kind="ExternalInput")
    wv_d = nc.dram_tensor("wv", [DIM, DIM], F16, kind="ExternalInput")
    wp_d = nc.dram_tensor("wp", [DIM, DIM], F16, kind="ExternalInput")
    wp2_d = nc.dram_tensor("wp2", [DIM, DIM], F16, kind="ExternalInput")
    sw1_d = nc.dram_tensor("sw1", [DIM, HID], F16, kind="ExternalInput")
    sw1q_d = nc.dram_tensor("sw1q", [DIM, HID], F16, kind="ExternalInput")
    xsum_d = nc.dram_tensor("xsum", [DIM, BC], F16, kind="ExternalInput")
    msd_d = nc.dram_tensor("msd", [N, BC], F16, kind="ExternalInput")
    sw2_d = nc.dram_tensor("sw2", [HID, DIM], F16, kind="ExternalInput")
    tm_d = nc.dram_tensor("tm", [N, N], F16, kind="ExternalInput")
    tx_d = nc.dram_tensor("tx", [N, N], F16, kind="ExternalInput")
    cb_d = nc.dram_tensor("cb", [1, 1], F32, kind="ExternalInput")
    scr_d = nc.dram_tensor("scr", [BC, N], F16, kind="ExternalOutput")
    outs_d = {
        nm: nc.dram_tensor(nm, [NT, DIM], F16, kind="ExternalOutput")
        for nm in ("x1", "y1", "xo", "yo")
    }

    with tile.TileContext(nc) as tc:
        _body(nc, tc, xT_d, yT_d, wq_d, wk_d, wv_d, wp_d, wp2_d,
              sw1_d, sw1q_d, xsum_d, msd_d, sw2_d, tm_d, tx_d, cb_d, scr_d, outs_d)
    nc.compile()
    return nc

def _body(nc, tc, xT_d, yT_d, wq_d, wk_d, wv_d, wp_d, wp2_d,
          sw1_d, sw1q_d, xsum_d, msd_d, sw2_d, tm_d, tx_d, cb_d, scr_d, outs_d):
    from contextlib import ExitStack
    from collections import deque
    from itertools import chain

    est = ExitStack()
    with est:
        const = est.enter_context(tc.tile_pool(name="const", bufs=1))
        id16 = const.tile([128, 128], F16, tag="id16", name="id16")
        make_identity(nc, id16)
        idbf = const.tile([128, 128], BF16, tag="idbf", name="idbf")
        make_identity(nc, idbf)
        cb72 = const.tile([MC, 1], F32, tag="cb72", name="cb72")
        nc.sync.dma_start(out=cb72, in_=cb_d[:, :].to_broadcast((MC, 1)))

        # persistent activation tiles
        act = est.enter_context(tc.tile_pool(name="act", bufs=1))
        qx6 = [act.tile([128, NT], F16, tag=f"qx{c}", name=f"qx{c}") for c in range(CH)]
        qy6 = [act.tile([128, NT], F16, tag=f"qy{c}", name=f"qy{c}") for c in range(CH)]
        ky6 = [act.tile([128, NT], F16, tag=f"ky{c}", name=f"ky{c}") for c in range(CH)]
        v16 = [act.tile([MC, 2 * 780], F16, tag=f"v16_{b}", name=f"v16_{b}")
               for b in range(BC)]
        zTy = act.tile([128, CH * NT], F16, tag="zTy", name="zTy")
        wp16 = act.tile([128, CH * DIM], F16, tag="wp16", name="wp16")
        wp216 = act.tile([128, CH * DIM], F16, tag="wp216", name="wp216")
        sa_pool = est.enter_context(tc.tile_pool(name="sa", bufs=1))
        se_pool = est.enter_context(tc.tile_pool(name="se", bufs=1))
        # qkv inputs/weights (persistent: x GEMMs weave into the y region)
        qkw = est.enter_context(tc.tile_pool(name="qkw", bufs=1))
        xt16 = [qkw.tile([128, 2 * NT], F16, tag=f"xt{kp}", name=f"xt{kp}")
                for kp in range(3)]
        yt16 = [qkw.tile([128, 2 * NT], F16, tag=f"yt{kp}", name=f"yt{kp}")
                for kp in range(3)]
        w16 = {w: qkw.tile([128, CH * DIM], F16, tag=f"w{w}", name=f"w{w}")
               for w in ("q", "k", "v")}
        vstage = [qkw.tile([128, HEADS * 65], F16, tag=f"vs{t}",
                           name=f"vs{t}") for t in range(9)]

        # aliases: kx lives in yt16's space (yt dead after ky GEMM);
        # zTx lives in xt16's space (xt dead after woven v/qx/kx GEMMs).
        def half_views(tiles):
            out = []
            for c in range(CH):
                out.append(tiles[c // 2][:, (c % 2) * NT:(c % 2 + 1) * NT])
            return out
        kx6 = half_views(yt16)
        zTx6 = half_views(xt16)
        zTy6 = [zTy.rearrange("p (c n) -> p c n", c=CH)[:, c, :]
                for c in range(CH)]

        evrot = [0]

        def evict(dst, src, rot="va"):
            """rotate psum evictions across DVE/ACT"""
            r = rot[evrot[0] % len(rot)]
            evrot[0] += 1
            if r == "a":
                nc.scalar.copy(dst, src)
            else:
                nc.vector.tensor_copy(dst, src)

        def load_w(wname, w_d):
            nc.sync.dma_start(
                out=w16[wname].rearrange("p (kc d) -> p kc d", kc=CH),
                in_=w_d.rearrange("(kc p) d -> p kc d", p=128))

        def load_in(src_d, dst):
            for kp in range(3):
                nc.sync.dma_start(
                    out=dst[kp].rearrange("p (i n) -> p i n", i=2),
                    in_=src_d[kp * 256:(kp + 1) * 256, :].rearrange(
                        "(i p) n -> p i n", i=2))

        # load order = first-use order
        load_w("q", wq_d)
        load_in(yT_d, yt16)
        load_w("k", wk_d)
        load_in(xT_d, xt16)
        load_w("v", wv_d)
        nc.sync.dma_start(
            out=wp16.rearrange("p (kc d) -> p kc d", kc=CH),
            in_=wp_d.rearrange("(kc p) d -> p kc d", p=128))
        nc.sync.dma_start(
            out=wp216.rearrange("p (kc d) -> p kc d", kc=CH),
            in_=wp2_d.rearrange("(kc p) d -> p kc d", p=128))

        def qkv_chunk(psum, wname, src, dst6, m, nf, rot, tag="qkv"):
            ps = psum.tile([128, NF], F32, tag=tag, name=tag)
            wv = w16[wname].rearrange("p (kc d) -> p kc d", kc=CH)
            for kc in range(CH):
                kp, i = kc // 2, kc % 2
                mov = src[kp].rearrange("p (i n) -> p i n", i=2)[
                    :, i, nf * NF:(nf + 1) * NF]
                nc.tensor.matmul(ps, wv[:, kc, m * 128:(m + 1) * 128], mov,
                                 start=(kc == 0), stop=(kc == CH - 1))
            evict(dst6[m][:, nf * NF:(nf + 1) * NF], ps, rot)

        def qkv_gemm(psum, wname, src, dst6, rot="va"):
            for m in range(CH):
                for nf in range(3):
                    qkv_chunk(psum, wname, src, dst6, m, nf, rot)

        with tc.tile_pool(name="qkvps", bufs=4, space="PSUM") as qkv_ps:
            qkv_gemm(qkv_ps, "q", yt16, qy6)

            # ---------------- SA gate part A (mean from host msd) -------
            accm = sa_pool.tile([128, NT], BF16, tag="accm", name="accm")
            nc.vector.tensor_max(accm, qy6[0], qy6[1])
            for c in range(2, CH):
                nc.vector.tensor_max(accm, accm, qy6[c])
            ms_a = sa_pool.tile([128, BC], F16, tag="msa", name="msa")
            ms_b = sa_pool.tile([16, BC], F16, tag="msb", name="msb")
            mx_a = sa_pool.tile([128, BC], F16, tag="mxa", name="mxa")
            mx_b = sa_pool.tile([16, BC], F16, tag="mxb", name="mxb")
            nc.sync.dma_start(out=ms_a, in_=msd_d[0:128, :])
            nc.sync.dma_start(out=ms_b, in_=msd_d[128:144, :])
            with tc.tile_pool(name="satp", bufs=1, space="PSUM") as satp:
                pa = satp.tile([128, BC * 128], BF16, tag="pa", name="pa")
                pb = satp.tile([16, BC * 128], BF16, tag="pb", name="pb")
                for b in range(BC):
                    nc.tensor.transpose(
                        pa[:, b * 128:(b + 1) * 128],
                        accm[:, b * N:b * N + 128], idbf)
                    nc.tensor.transpose(
                        pb[:, b * 128:(b + 1) * 128],
                        accm[:, b * N + 128:(b + 1) * N], idbf)
                with nc.allow_low_precision(reason="SA gate pooling"):
                    nc.vector.reduce_max(
                        mx_a, pa.rearrange("p (b n) -> p b n", n=128), axis=AX.X)
                    nc.vector.reduce_max(
                        mx_b, pb.rearrange("p (b n) -> p b n", n=128), axis=AX.X)
                # toeplitz conv + sigmoid gate
                tm_a = sa_pool.tile([128, N], F16, tag="tma", name="tma")
                tm_b = sa_pool.tile([16, N], F16, tag="tmb", name="tmb")
                tx_a = sa_pool.tile([128, N], F16, tag="txa", name="txa")
                tx_b = sa_pool.tile([16, N], F16, tag="txb", name="txb")
                nc.sync.dma_start(out=tm_a, in_=tm_d[0:128, :])
                nc.sync.dma_start(out=tm_b, in_=tm_d[128:144, :])
                nc.sync.dma_start(out=tx_a, in_=tx_d[0:128, :])
                nc.sync.dma_start(out=tx_b, in_=tx_d[128:144, :])
                tg = [sa_pool.tile([MC, BC], F16, tag=f"tg{h}", name=f"tg{h}")
                      for h in range(2)]
                for half in range(2):
                    tp = satp.tile([MC, BC], F32, tag="tp", name="tp")
                    sl = slice(half * MC, (half + 1) * MC)
                    for i, (tmat, mv) in enumerate((
                            (tm_a, ms_a), (tm_b, ms_b),
                            (tx_a, mx_a), (tx_b, mx_b))):
                        nc.tensor.matmul(tp, tmat[:, sl], mv,
                                         start=(i == 0), stop=(i == 3))
                    nc.scalar.activation(tg[half], tp, AF.Sigmoid, bias=cb72)
                for half in range(2):
                    nc.sync.dma_start(
                        out=scr_d.rearrange("b n -> n b")[
                            half * MC:(half + 1) * MC, :],
                        in_=tg[half])
                trow = sa_pool.tile([1, NT], F16, tag="trow", name="trow")
                nc.sync.dma_start(
                    out=trow,
                    in_=scr_d.rearrange("b n -> (b n)").unsqueeze(0))
                tbc = sa_pool.tile([128, NT], F16, tag="tbc", name="tbc")
                nc.gpsimd.partition_broadcast(tbc, trow, channels=128)

            qkv_gemm(qkv_ps, "k", yt16, ky6, rot="av")

            # SA part B: gate qy
            for c in range(CH):
                nc.vector.scalar_tensor_tensor(
                    qy6[c], tbc, 1.0, qy6[c], op0=ALU.add, op1=ALU.mult)

            # x column-sums (host) for SE mean path
            xsum = [se_pool.tile([128, 16], F16, tag=f"xs{kp}",
                                 name=f"xs{kp}") for kp in range(3)]
            for kp in range(3):
                nc.sync.dma_start(
                    out=xsum[kp].rearrange("p (i b) -> p i b", i=2),
                    in_=xsum_d[kp * 256:(kp + 1) * 256, :].rearrange(
                        "(i p) b -> p i b", i=2))

        # ------------------------------------------------ attention + proj
        proj_sched = {b: [b] for b in range(BC)}
        proj_sched[7] = [7, 8]

        with tc.tile_pool(name="aps", bufs=2, space="PSUM") as s_ps, \
             tc.tile_pool(name="avp", bufs=2, space="PSUM") as av_ps, \
             tc.tile_pool(name="ztps", bufs=1, space="PSUM") as zt_ps, \
             tc.tile_pool(name="seps", bufs=1, space="PSUM") as se_ps, \
             tc.tile_pool(name="pps", bufs=2, space="PSUM") as p_ps, \
             tc.tile_pool(name="es", bufs=2) as es_pool, \
             tc.tile_pool(name="zt", bufs=3) as zt_pool, \
             tc.tile_pool(name="nrm", bufs=2) as nrm_pool, \
             tc.tile_pool(name="ostg", bufs=4) as ostg_pool:

            def v_ops():
                """V GEMM t-chunks + per-batch repacks, emission-ordered so
                repack(b) lands before av(b) is woven (block b+1)."""
                done_t = [0]

                def vchunk(t, half, part):
                    def op(ps=[None]):
                        wv = w16["v"].rearrange("p (kc d) -> p kc d", kc=CH)
                        if part == 0:
                            if half == 0:
                                ones = vstage[t].rearrange(
                                    "p (h o) -> p h o", o=65)[:, :, 64:65]
                                nc.vector.memset(ones, 1.0)
                            vps[0] = p_ps.tile([128, NF], F32, tag="pp",
                                               name="pp")
                        rng = range(3) if part == 0 else range(3, CH)
                        for kc in rng:
                            kp, i = kc // 2, kc % 2
                            stat = xt16[kp].rearrange(
                                "p (i n) -> p i n", i=2)[
                                :, i, t * 128:(t + 1) * 128]
                            nc.tensor.matmul(
                                vps[0], stat,
                                wv[:, kc, half * NF:(half + 1) * NF],
                                start=(kc == 0), stop=(kc == CH - 1))
                        if part == 1:
                            dst3 = vstage[t].rearrange(
                                "p (h o) -> p h o", o=65)[
                                :, half * 6:(half + 1) * 6, 0:64]
                            evict(dst3,
                                  vps[0].rearrange("p (h d) -> p h d", d=64),
                                  rot="v")
                    return op

                vps = [None]

                def repack(b):
                    def op():
                        for j in range(2):
                            row0 = b * N + j * MC
                            pos = 0
                            while pos < MC:
                                t = (row0 + pos) // 128
                                r0 = (row0 + pos) % 128
                                cnt = min(128 - r0, MC - pos)
                                nc.sync.dma_start(
                                    out=v16[b][pos:pos + cnt,
                                               j * 780:(j + 1) * 780],
                                    in_=vstage[t][r0:r0 + cnt, :])
                                pos += cnt
                    return op

                nextb = 0
                for t in range(9):
                    for half in range(2):
                        yield 480, vchunk(t, half, 0)
                        yield 480, vchunk(t, half, 1)
                    done_t[0] = t + 1
                    while nextb < BC and (nextb + 1) * N <= (t + 1) * 128:
                        yield 0, repack(nextb)
                        nextb += 1
                while nextb < BC:
                    yield 0, repack(nextb)
                    nextb += 1

            def proj_ops(t, zT6, pairs, erot="v"):
                """(pe_cost_ns, thunk) pieces for one proj t-chunk"""
                for wt, od in pairs:
                    wv = wt.rearrange("p (kc d) -> p kc d", kc=CH)
                    stage = ostg_pool.tile([128, DIM], F16, tag="ostg",
                                           name="ostg")
                    for nf in range(2):
                        ps = p_ps.tile([128, NF], F32, tag="pp", name="pp")
                        for kc in range(CH):
                            def mm(ps=ps, nf=nf, kc=kc):
                                nc.tensor.matmul(
                                    ps, zT6[kc][:, t * 128:(t + 1) * 128],
                                    wv[:, kc, nf * NF:(nf + 1) * NF],
                                    start=(kc == 0), stop=(kc == CH - 1))
                            yield 160, mm
                        def ev(ps=ps, nf=nf, stage=stage):
                            dst = stage[:, nf * NF:(nf + 1) * NF]
                            evict(dst, ps, rot=erot)
                        yield 0, ev
                    def dma(od=od, stage=stage):
                        nc.sync.dma_start(out=od[t * 128:(t + 1) * 128, :],
                                          in_=stage)
                    yield 0, dma

            def qkv_ops(wname, src, dst6, rot):
                wv = w16[wname].rearrange("p (kc d) -> p kc d", kc=CH)
                for m in range(CH):
                    for nf in range(3):
                        ps = p_ps.tile([128, NF], F32, tag="pp", name="pp")
                        def mm1(ps=ps, m=m, nf=nf):
                            for kc in range(3):
                                kp, i = kc // 2, kc % 2
                                mov = src[kp].rearrange(
                                    "p (i n) -> p i n", i=2)[
                                    :, i, nf * NF:(nf + 1) * NF]
                                nc.tensor.matmul(
                                    ps, wv[:, kc, m * 128:(m + 1) * 128],
                                    mov, start=(kc == 0), stop=False)
                        def mm2(ps=ps, m=m, nf=nf):
                            for kc in range(3, CH):
                                kp, i = kc // 2, kc % 2
                                mov = src[kp].rearrange(
                                    "p (i n) -> p i n", i=2)[
                                    :, i, nf * NF:(nf + 1) * NF]
                                nc.tensor.matmul(
                                    ps, wv[:, kc, m * 128:(m + 1) * 128],
                                    mov, start=False, stop=(kc == CH - 1))
                            evict(dst6[m][:, nf * NF:(nf + 1) * NF], ps, rot)
                        yield 480, mm1
                        yield 480, mm2

            def se_ops():
                """SE gate chain as weave pieces (needs all qx6 written)"""
                sw1t = se_pool.tile([128, CH * HID], F16, tag="sw1", name="sw1")
                sw1qt = se_pool.tile([128, CH * HID], F16, tag="sw1q",
                                     name="sw1q")
                sw2t = se_pool.tile([HID, DIM], F16, tag="sw2", name="sw2")
                maxs = [se_pool.tile([128, BC], F16, tag=f"max{c}",
                                     name=f"max{c}") for c in range(CH)]

                def ld():
                    nc.sync.dma_start(
                        out=sw1t.rearrange("p (kc h) -> p kc h", kc=CH),
                        in_=sw1_d.rearrange("(kc p) h -> p kc h", p=128))
                    nc.sync.dma_start(
                        out=sw1qt.rearrange("p (kc h) -> p kc h", kc=CH),
                        in_=sw1q_d.rearrange("(kc p) h -> p kc h", p=128))
                    nc.sync.dma_start(out=sw2t, in_=sw2_d[:, :])
                yield 0, ld
                for c in range(CH):
                    def red(c=c):
                        q3 = qx6[c].rearrange("p (b n) -> p b n", n=N)
                        with nc.allow_low_precision(reason="SE pooling"):
                            nc.vector.reduce_max(maxs[c], q3, axis=AX.X)
                    yield 0, red
                hids = []
                def fc1m():
                    ps = se_ps.tile([128, NF], F32, tag="sep", name="sep")
                    fc1p = ps[0:HID, 0:BC]
                    sw1qv = sw1qt.rearrange("p (kc h) -> p kc h", kc=CH)
                    first = True
                    for kp in range(3):
                        xs3 = xsum[kp].rearrange("p (i b) -> p i b", i=2)
                        for i in range(2):
                            nc.tensor.matmul(fc1p, sw1qv[:, kp * 2 + i, :],
                                             xs3[:, i, :], start=first,
                                             stop=(kp == 2 and i == 1))
                            first = False
                    hid = se_pool.tile([HID, BC], F16, tag="hidm",
                                       name="hidm")
                    nc.scalar.activation(hid, fc1p, AF.Relu, scale=1.0 / N)
                    hids.append(hid)
                yield 100, fc1m
                def fc1x():
                    ps = se_ps.tile([128, NF], F32, tag="sep", name="sep")
                    fc1p = ps[0:HID, 0:BC]
                    sw1v = sw1t.rearrange("p (kc h) -> p kc h", kc=CH)
                    for c in range(CH):
                        nc.tensor.matmul(fc1p, sw1v[:, c, :], maxs[c],
                                         start=(c == 0), stop=(c == CH - 1))
                    hid = se_pool.tile([HID, BC], F16, tag="hidx",
                                       name="hidx")
                    nc.scalar.activation(hid, fc1p, AF.Relu)
                    hids.append(hid)
                yield 100, fc1x
                for c in range(CH):
                    def fc2(c=c):
                        sgs = []
                        for pi in range(2):
                            ps = se_ps.tile([128, NF], F32, tag="sep",
                                            name="sep")
                            ps2 = ps[:, 0:BC]
                            nc.tensor.matmul(ps2,
                                             sw2t[:, c * 128:(c + 1) * 128],
                                             hids[pi], start=True, stop=True)
                            sg = se_pool.tile([128, BC], F16, tag=f"sg{pi}_{c}",
                                              name=f"sg{pi}_{c}")
                            nc.scalar.activation(sg, ps2, AF.Sigmoid)
                            sgs.append(sg)
                        g1 = se_pool.tile([128, BC], F16, tag=f"g1{c}",
                                          name=f"g1{c}")
                        nc.vector.scalar_tensor_tensor(
                            g1, sgs[0], 1.0, sgs[1], op0=ALU.add, op1=ALU.add)
                        q3 = qx6[c].rearrange("p (b n) -> p b n", n=N)
                        g3 = g1.unsqueeze(2).to_broadcast((128, BC, N))
                        nc.vector.tensor_tensor(q3, q3, g3, op=ALU.mult)
                    yield 60, fc2

            def av_ops(b, e16, ztgroups):
                """av + normalize + zT for one finished block"""
                col0 = b * N
                for i in range(2):
                    def grp(i=i):
                        zt = zt_pool.tile([MC, DIM], F16, tag="zt", name="zt")
                        rec = nrm_pool.tile([MC, HEADS], F32, tag="rec",
                                            name="rec")
                        for half in range(2):
                            oaug = av_ps.tile([MC, 6 * 65], F32, tag="oa",
                                              name="oa")
                            for hh in range(6):
                                h = half * 6 + hh
                                for j in range(2):
                                    lhs = e16[h][:, j * N + i * MC:
                                                 j * N + (i + 1) * MC]
                                    rhs = v16[b][:, j * 780 + h * 65:
                                                 j * 780 + (h + 1) * 65]
                                    nc.tensor.matmul(
                                        oaug[:, hh * 65:(hh + 1) * 65],
                                        lhs, rhs, start=(j == 0),
                                        stop=(j == 1))
                            o3 = oaug.rearrange("p (h o) -> p h o", o=65)
                            rsl = rec[:, half * 6:(half + 1) * 6]
                            nc.vector.reciprocal(rsl, o3[:, :, 64:65])
                            z3 = zt.rearrange("p (h d) -> p h d", d=64)[
                                :, half * 6:(half + 1) * 6, :]
                            r3 = rsl.unsqueeze(2).to_broadcast((MC, 6, 64))
                            nc.vector.tensor_tensor(z3, o3[:, :, 0:64],
                                                    r3, op=ALU.mult)
                        ztp = zt_ps.tile([128, CH * MC], F16, tag="ztp",
                                         name="ztp")
                        for c in range(CH):
                            nc.tensor.transpose(ztp[:, c * MC:(c + 1) * MC],
                                                zt[:, c * 128:(c + 1) * 128],
                                                id16[0:MC, 0:MC])
                        z3p = ztp.rearrange("p (c n) -> p c n", c=CH)
                        ecol = col0 + i * MC
                        for view3, cnt, c0 in ztgroups:
                            evict(view3[:, :, ecol:ecol + MC],
                                  z3p[:, c0:c0 + cnt, :], rot="v")
                    yield 450, grp

            urgent = deque()
            prep = [iter(())]
            bulk = [iter(())]

            def drip(budget):
                while urgent:
                    cost, op = urgent.popleft()
                    op()
                    budget -= max(cost, 20)
                while budget > 0:
                    cost_op = next(prep[0], None)
                    if cost_op is None:
                        cost_op = next(bulk[0], None)
                        if cost_op is None:
                            return
                    cost, op = cost_op
                    op()
                    budget -= max(cost, 20)

            def s_exp_block(b, qq, kk):
                col0 = b * N
                e16 = []
                for h in range(HEADS):
                    c6 = h // 2
                    p0 = (h % 2) * 64
                    q_ap = qq[c6][p0:p0 + 64, col0:col0 + N]
                    sps = s_ps.tile([MC, 2 * N], F32, tag="S", name="S")
                    for j in range(2):
                        k_ap = kk[c6][p0:p0 + 64,
                                      col0 + j * MC:col0 + (j + 1) * MC]
                        nc.tensor.matmul(sps[:, j * N:(j + 1) * N],
                                         k_ap, q_ap, start=True, stop=True)
                    e = es_pool.tile([MC, 2 * N], F16, tag=f"e16_{h}",
                                     name=f"e16_{h}")
                    nc.scalar.activation(e, sps, AF.Exp, scale=ESC)
                    e16.append(e)
                    drip(350)
                return e16

            # merged block order: first x blocks slot into y's ACT tail.
            # proj chunks enter the weave two blocks after their zT rows
            # land, shifting PE filler toward ACT-bound stretches.
            ztg_y = [(zTy.rearrange("p (c n) -> p c n", c=CH), CH, 0)]
            ztg_x = [(xt16[kp].rearrange("p (i n) -> p i n", i=2), 2, kp * 2)
                     for kp in range(3)]
            cfg = {"y": (qy6, ky6, zTy6, "y1", "yo", ztg_y),
                   "x": (qx6, kx6, zTx6, "x1", "xo", ztg_x)}
            order = ([("y", b) for b in range(6)]
                     + [("x", 0), ("y", 6), ("x", 1), ("y", 7)]
                     + [("x", b) for b in range(2, BC)])
            lag = {"y": deque(), "x": deque()}

            def queue_proj(sd, pb):
                qq, kk, zT6, o1, o2, _ztg = cfg[sd]
                erot = "vva"
                for t in proj_sched[pb]:
                    bulk[0] = chain(bulk[0], proj_ops(
                        t, zT6, ((wp16, outs_d[o1]), (wp216, outs_d[o2])),
                        erot=erot))

            # x-side prep woven into the y region
            prep[0] = chain(v_ops(),
                            qkv_ops("q", xt16, qx6, "va"),
                            se_ops(),
                            qkv_ops("k", xt16, kx6, "va"))
            prev = None
            for sd, b in order:
                if prev is not None:
                    psd, pb, pe = prev
                    urgent.extend(av_ops(pb, pe, cfg[psd][5]))
                    lag[psd].append(pb)
                    if len(lag[psd]) >= 2:
                        queue_proj(psd, lag[psd].popleft())
                if (sd, b) == ("x", 0):
                    # x blocks read qx/kx/SE outputs; emit any remaining
                    # prep pieces now (normally already drained)
                    for cost, op in prep[0]:
                        op()
                    prep[0] = iter(())
                e16 = s_exp_block(b, *cfg[sd][:2])
                prev = (sd, b, e16)
            psd, pb, pe = prev
            urgent.extend(av_ops(pb, pe, cfg[psd][5]))
            lag[psd].append(pb)
            for sd in ("y", "x"):
                while lag[sd]:
                    queue_proj(sd, lag[sd].popleft())
            while urgent:
                urgent.popleft()[1]()
            for cost, op in bulk[0]:
                op()



def _f8(a):
    return np.clip(a, -240.0, 240.0).astype(ml_dtypes.float8_e4m3)


def _toeplitz(k5):
    """[144,144] T with T[m,n] = k5[my-ny+2, mx-nx+2]"""
    t = np.zeros((N, N), np.float64)
    for ny in range(PATCH):
        for nx in range(PATCH):
            for dy in range(-2, 3):
                for dx in range(-2, 3):
                    my, mx = ny + dy, nx + dx
                    if 0 <= my < PATCH and 0 <= mx < PATCH:
                        t[my * PATCH + mx, ny * PATCH + nx] = k5[dy + 2, dx + 2]
    return t


def _prep_weights(inputs):
    Wqkv = np.asarray(inputs["Wqkv"], np.float64)
    wq = Wqkv[:, DIM:2 * DIM].astype(np.float16)
    wk = Wqkv[:, 2 * DIM:3 * DIM].astype(np.float16)
    wv = Wqkv[:, 3 * DIM:4 * DIM].astype(np.float16)
    wp64 = np.asarray(inputs["Wproj"], np.float64)
    wp = wp64.astype(np.float16)
    wp2 = (wp64 @ wp64).astype(np.float16)
    bp = np.asarray(inputs["bproj"], np.float64).reshape(1, DIM)
    bp2 = (bp @ wp64 + bp).astype(np.float32)
    sw1_64 = np.asarray(inputs["se_w1"], np.float64)
    sw1 = sw1_64.astype(np.float16)
    wq64 = Wqkv[:, DIM:2 * DIM]
    sw1q = (wq64 @ sw1_64).astype(np.float16)
    wqmean = wq64.mean(axis=1)
    sw2 = np.asarray(inputs["se_w2"], np.float16)
    sa_w = np.asarray(inputs["sa_w"], np.float64)
    tm = _toeplitz(sa_w[0, 0]).astype(np.float16)
    tx = _toeplitz(sa_w[0, 1]).astype(np.float16)
    cb = np.asarray(inputs["sa_b"], np.float32).reshape(1, 1)
    return dict(wq=wq, wk=wk, wv=wv, wp=wp, wp2=wp2, sw1=sw1, sw1q=sw1q,
                sw2=sw2, tm=tm, tx=tx, cb=cb), \
        bp.astype(np.float32), bp2, wqmean


def kernel(**inputs):
    from concourse.bass_utils import run_bass_kernel_spmd

    if "nc" not in _COMPILED:
        _COMPILED["nc"] = build_program()
    nc = _COMPILED["nc"]

    w, bp, bp2, wqmean = _prep_weights(inputs)
    x = np.asarray(inputs["x"], np.float32).reshape(B, N, DIM)
    y = np.asarray(inputs["y"], np.float32).reshape(B, N, DIM)
    in_maps = []
    for i in range(NCORES):
        m = dict(w)
        m["xT"] = np.ascontiguousarray(
            x[i * BC:(i + 1) * BC].reshape(NT, DIM).T).astype(np.float16)
        ysh = y[i * BC:(i + 1) * BC].reshape(NT, DIM)
        m["yT"] = np.ascontiguousarray(ysh.T).astype(np.float16)
        xsh = x[i * BC:(i + 1) * BC].reshape(BC, N, DIM)
        m["xsum"] = np.ascontiguousarray(
            xsh.sum(axis=1, dtype=np.float64).T).astype(np.float16)
        m["msd"] = np.ascontiguousarray(
            (ysh.astype(np.float64) @ wqmean).reshape(BC, N).T
        ).astype(np.float16)
        in_maps.append(m)

    res = run_bass_kernel_spmd(nc, in_maps, core_ids=list(range(NCORES)))
    outs = []
    for name, bias in (("x1", bp), ("y1", bp), ("xo", bp2), ("yo", bp2)):
        full = np.concatenate(
            [np.asarray(res.results[i][name], np.float32).reshape(BC, N, DIM)
             for i in range(NCORES)], axis=0)
        outs.append(full + bias.reshape(1, 1, DIM))
    return tuple(outs)



# revision 15
# speedup vs baseline: 1.1136x; 1.1136x over previous
"""Trainium2 Bass kernel for nn_Attention_29326036697657 (sparse_attention).

Dual-input attention with SE (channel) / SA (spatial) gates.
Sharding: data-parallel over batch B=64 across 8 cores (8 batches/core).

Algebraic folds (vs reference):
  - qxo/qyo/attnx dead -> Wqkv comp 0 unused; vy = vx (reference quirk).
  - dots(q,k)+dots(q2,k) = dots(q*(1+g), k) for both SE and SA gates.
  - softmax without max-subtraction (logits O(1)); denominator via a
    ones column appended to each V head block.
  - bias + scale fixups applied on HOST after gather.
  - SA 5x5 conv lowered to two host-built Toeplitz matmuls (TM/768, TX).

Numerics: the five qkv GEMMs and four proj GEMMs run as split-fp8
DoubleRow matmuls (residual decomposition a=ah+al, w=wh+wl, dropping
the al*wl term): 3 chains x 3 DR steps of K=256 each at 0.5 cyc/row
vs fp16's 6 steps at 1.0 -- 25% fewer PE cycles at ~1e-3 accuracy.
Weights are host-scaled x64 so fp8 residuals clear the subnormal
floor; activations carry x64 which folds into the exp scale
(ESC/4096), the two sigmoid scales (1/64), and a host-side /4096.
S and AV GEMMs stay fp16 (single-fp8 q/k measured >2e-2). zT is
split to fp8 hi+lo on eviction (one extra DVE pass per block).

Layout: inputs arrive HOST-pre-quantized ([768, NT] fp8 hi+lo) so the
kernel does zero input transposes/quantization. q/k tiles [128, NT]
f16; v natural per-batch [72, 2*780] f16 with denominator columns; z
transposed back via PE (f16 identity); proj reads fp8 zT slabs, psum
evicted f16 and DMA'd per t-chunk.

Schedule: software-pipelined around the ACT exp chain: attention
S+exp blocks drip a cost-budgeted weave of V/qk-GEMMs, the SE chain,
and projections (2-block lag) into exp-wait gaps; x-side blocks
interleave into the y-side ACT tail. kx aliases the dead y-input fp8
tiles (bitcast), zTx h/l alias the dead x-input fp8 tiles. GPSIMD
cannot touch PSUM, so evictions rotate across DVE/ACT only.
"""

import sys

sys.path.insert(0, "/opt/trn_rl_repo")

import numpy as np
import ml_dtypes

import concourse.bass as bass
import concourse.bacc as bacc_mod
import concourse.mybir as mybir
import concourse.tile as tile
from concourse.masks import make_identity

# ---------------------------------------------------------------- constants
DIM = 768
HEADS = 12
PATCH = 12
N = PATCH * PATCH          # 144
B = 64
RED = 16
HID = DIM // RED           # 48
HD = DIM // HEADS          # 64
SCALE = HD ** -0.5         # 0.125

NCORES = 8
BC = B // NCORES           # 8 batches per core
NT = BC * N                # 1152 rows per core
CH = DIM // 128            # 6 channel chunks
NF = 384                   # qkv/proj moving chunk
MC = 72                    # m/n half within one batch
WS = 64.0                  # host weight scale (fp8 sweet spot)
ESC = SCALE / (WS * WS)    # exp scale (q,k each carry x64)
GS = 1.0 / WS              # gate sigmoid input unscale

F32 = mybir.dt.float32
F16 = mybir.dt.float16
BF16 = mybir.dt.bfloat16
F8 = mybir.dt.float8e4
AX = mybir.AxisListType
AF = mybir.ActivationFunctionType
ALU = mybir.AluOpType
DR = mybir.MatmulPerfMode.DoubleRow

_COMPILED = {}


def build_program():
    nc = bacc_mod.Bacc()

    xin_d = {s: nc.dram_tensor(f"x{s}", [DIM, NT], F8, kind="ExternalInput")
             for s in "hl"}
    yin_d = {s: nc.dram_tensor(f"y{s}", [DIM, NT], F8, kind="ExternalInput")
             for s in "hl"}
    w8_d = {(w, s): nc.dram_tensor(f"w{w}{s}", [DIM, DIM], F8,
                                   kind="ExternalInput")
            for w in "qkv" for s in "hl"}
    wp8_d = {(p, s): nc.dram_tensor(f"wp{p}{s}", [DIM, DIM], F8,
                                    kind="ExternalInput")
             for p in (1, 2) for s in "hl"}
    sw1_d = nc.dram_tensor("sw1", [DIM, HID], F16, kind="ExternalInput")
    sw1q_d = nc.dram_tensor("sw1q", [DIM, HID], F16, kind="ExternalInput")
    xsum_d = nc.dram_tensor("xsum", [DIM, BC], F16, kind="ExternalInput")
    msd_d = nc.dram_tensor("msd", [N, BC], F16, kind="ExternalInput")
    sw2_d = nc.dram_tensor("sw2", [HID, DIM], F16, kind="ExternalInput")
    tm_d = nc.dram_tensor("tm", [N, N], F16, kind="ExternalInput")
    tx_d = nc.dram_tensor("tx", [N, N], F16, kind="ExternalInput")
    cb_d = nc.dram_tensor("cb", [1, 1], F32, kind="ExternalInput")
    scr_d = nc.dram_tensor("scr", [BC, N], F16, kind="ExternalOutput")
    outs_d = {
        nm: nc.dram_tensor(nm, [NT, DIM], F16, kind="ExternalOutput")
        for nm in ("x1", "y1", "xo", "yo")
    }

    with tile.TileContext(nc) as tc:
        _body(nc, tc, xin_d, yin_d, w8_d, wp8_d,
              sw1_d, sw1q_d, xsum_d, msd_d, sw2_d, tm_d, tx_d, cb_d, scr_d,
              outs_d)
    nc.compile()
    return nc

def _body(nc, tc, xin_d, yin_d, w8_d, wp8_d,
          sw1_d, sw1q_d, xsum_d, msd_d, sw2_d, tm_d, tx_d, cb_d, scr_d,
          outs_d):
    from contextlib import ExitStack
    from collections import deque
    from itertools import chain

    est = ExitStack()
    with est:
        const = est.enter_context(tc.tile_pool(name="const", bufs=1))
        id16 = const.tile([128, 128], F16, tag="id16", name="id16")
        make_identity(nc, id16)
        idbf = const.tile([128, 128], BF16, tag="idbf", name="idbf")
        make_identity(nc, idbf)
        cb72 = const.tile([MC, 1], F32, tag="cb72", name="cb72")
        nc.sync.dma_start(out=cb72, in_=cb_d[:, :].to_broadcast((MC, 1)))

        # persistent activation tiles
        act = est.enter_context(tc.tile_pool(name="act", bufs=1))
        qx6 = [act.tile([128, NT], F16, tag=f"qx{c}", name=f"qx{c}") for c in range(CH)]
        qy6 = [act.tile([128, NT], F16, tag=f"qy{c}", name=f"qy{c}") for c in range(CH)]
        ky6 = [act.tile([128, NT], F16, tag=f"ky{c}", name=f"ky{c}") for c in range(CH)]
        v16 = [act.tile([MC, 2 * 780], F16, tag=f"v16_{b}", name=f"v16_{b}")
               for b in range(BC)]
        zTy8 = {s: act.tile([128, CH * NT], F8, tag=f"zTy{s}", name=f"zTy{s}")
                for s in "hl"}
        wp8 = {k: act.tile([128, CH * DIM], F8, tag=f"wp{k[0]}{k[1]}",
                           name=f"wp{k[0]}{k[1]}")
               for k in wp8_d}
        sa_pool = est.enter_context(tc.tile_pool(name="sa", bufs=1))
        se_pool = est.enter_context(tc.tile_pool(name="se", bufs=1))
        # qkv inputs/weights (persistent: x GEMMs weave into the y region)
        qkw = est.enter_context(tc.tile_pool(name="qkw", bufs=1))
        xt8 = {s: [qkw.tile([128, 2 * NT], F8, tag=f"xt{s}{kp}",
                            name=f"xt{s}{kp}") for kp in range(3)]
               for s in "hl"}
        yt8 = {s: [qkw.tile([128, 2 * NT], F8, tag=f"yt{s}{kp}",
                            name=f"yt{s}{kp}") for kp in range(3)]
               for s in "hl"}
        w8 = {k: qkw.tile([128, CH * DIM], F8, tag=f"w{k[0]}{k[1]}",
                          name=f"w{k[0]}{k[1]}")
              for k in w8_d}
        vstage = [qkw.tile([128, HEADS * 65], F16, tag=f"vs{t}",
                           name=f"vs{t}") for t in range(9)]

        # aliases: kx (fp16) lives in the dead y fp8 tiles via bitcast;
        # zTx h/l live in the dead x fp8 tiles (same dtype slicing).
        kx6 = [(yt8["h"] + yt8["l"])[c].bitcast(F16) for c in range(CH)]
        # [128, 2, NT] chunk-pair views for the proj stationary reads
        zpx = {s: [xt8[s][kc2].rearrange("p (i n) -> p i n", i=2)
                   for kc2 in range(3)] for s in "hl"}
        zpy = {s: [zTy8[s].rearrange("p (c n) -> p c n", c=CH)[
                       :, 2 * kc2:2 * kc2 + 2, :]
                   for kc2 in range(3)] for s in "hl"}

        evrot = [0]

        def evict(dst, src, rot="va"):
            """rotate psum evictions across DVE/ACT"""
            r = rot[evrot[0] % len(rot)]
            evrot[0] += 1
            if r == "a":
                nc.scalar.copy(dst, src)
            else:
                nc.vector.tensor_copy(dst, src)

        def load_w(key, w_d, tiles):
            # one DMA per kc-pair so the first matmuls unblock early
            wv = tiles[key].rearrange("p (kc d) -> p kc d", kc=CH)
            for c2 in range(3):
                nc.sync.dma_start(
                    out=wv[:, 2 * c2:2 * c2 + 2, :],
                    in_=w_d[key][c2 * 256:(c2 + 1) * 256, :].rearrange(
                        "(i p) d -> p i d", p=128))

        def load_in(src_d, dst, s):
            for kp in range(3):
                nc.sync.dma_start(
                    out=dst[s][kp].rearrange("p (i n) -> p i n", i=2),
                    in_=src_d[s][kp * 256:(kp + 1) * 256, :].rearrange(
                        "(i p) n -> p i n", i=2))

        # load order = first-use order (h before l within each chain set)
        load_w(("q", "h"), w8_d, w8)
        load_in(yin_d, yt8, "h")
        load_w(("q", "l"), w8_d, w8)
        load_in(yin_d, yt8, "l")
        load_w(("k", "h"), w8_d, w8)
        load_w(("k", "l"), w8_d, w8)
        load_in(xin_d, xt8, "h")
        load_in(xin_d, xt8, "l")
        load_w(("v", "h"), w8_d, w8)
        load_w(("v", "l"), w8_d, w8)
        for k in wp8_d:
            load_w(k, wp8_d, wp8)

        CHAINS = (("h", "h"), ("h", "l"), ("l", "h"))

        def qkv_chunk(psum, wname, src, dst6, m, nf, rot, tag="qkv"):
            ps = psum.tile([128, NF], F32, tag=tag, name=tag)
            for ci, (sx, sw) in enumerate(CHAINS):
                wv = w8[(wname, sw)].rearrange("p (kc d) -> p kc d", kc=CH)
                for kp in range(3):
                    mov = src[sx][kp].rearrange("p (i n) -> p i n", i=2)[
                        :, :, nf * NF:(nf + 1) * NF]
                    nc.tensor.matmul(
                        ps, wv[:, 2 * kp:2 * kp + 2, m * 128:(m + 1) * 128],
                        mov, start=(ci == 0 and kp == 0),
                        stop=(ci == 2 and kp == 2), perf_mode=DR)
            evict(dst6[m][:, nf * NF:(nf + 1) * NF], ps, rot)

        def qkv_gemm(psum, wname, src, dst6, rot="va"):
            for m in range(CH):
                for nf in range(3):
                    qkv_chunk(psum, wname, src, dst6, m, nf, rot)

        with tc.tile_pool(name="qkvps", bufs=4, space="PSUM") as qkv_ps:
            qkv_gemm(qkv_ps, "q", yt8, qy6)

            # ---------------- SA gate part A (mean from host msd) -------
            accm = sa_pool.tile([128, NT], BF16, tag="accm", name="accm")
            nc.vector.tensor_max(accm, qy6[0], qy6[1])
            for c in range(2, CH):
                nc.vector.tensor_max(accm, accm, qy6[c])
            ms_a = sa_pool.tile([128, BC], F16, tag="msa", name="msa")
            ms_b = sa_pool.tile([16, BC], F16, tag="msb", name="msb")
            mx_a = sa_pool.tile([128, BC], F16, tag="mxa", name="mxa")
            mx_b = sa_pool.tile([16, BC], F16, tag="mxb", name="mxb")
            nc.sync.dma_start(out=ms_a, in_=msd_d[0:128, :])
            nc.sync.dma_start(out=ms_b, in_=msd_d[128:144, :])
            with tc.tile_pool(name="satp", bufs=1, space="PSUM") as satp:
                pa = satp.tile([128, BC * 128], BF16, tag="pa", name="pa")
                pb = satp.tile([16, BC * 128], BF16, tag="pb", name="pb")
                for b in range(BC):
                    nc.tensor.transpose(
                        pa[:, b * 128:(b + 1) * 128],
                        accm[:, b * N:b * N + 128], idbf)
                    nc.tensor.transpose(
                        pb[:, b * 128:(b + 1) * 128],
                        accm[:, b * N + 128:(b + 1) * N], idbf)
                with nc.allow_low_precision(reason="SA gate pooling"):
                    nc.vector.reduce_max(
                        mx_a, pa.rearrange("p (b n) -> p b n", n=128), axis=AX.X)
                    nc.vector.reduce_max(
                        mx_b, pb.rearrange("p (b n) -> p b n", n=128), axis=AX.X)
                # toeplitz conv + sigmoid gate
                tm_a = sa_pool.tile([128, N], F16, tag="tma", name="tma")
                tm_b = sa_pool.tile([16, N], F16, tag="tmb", name="tmb")
                tx_a = sa_pool.tile([128, N], F16, tag="txa", name="txa")
                tx_b = sa_pool.tile([16, N], F16, tag="txb", name="txb")
                nc.sync.dma_start(out=tm_a, in_=tm_d[0:128, :])
                nc.sync.dma_start(out=tm_b, in_=tm_d[128:144, :])
                nc.sync.dma_start(out=tx_a, in_=tx_d[0:128, :])
                nc.sync.dma_start(out=tx_b, in_=tx_d[128:144, :])
                tg = [sa_pool.tile([MC, BC], F16, tag=f"tg{h}", name=f"tg{h}")
                      for h in range(2)]
                for half in range(2):
                    tp = satp.tile([MC, BC], F32, tag="tp", name="tp")
                    sl = slice(half * MC, (half + 1) * MC)
                    for i, (tmat, mv) in enumerate((
                            (tm_a, ms_a), (tm_b, ms_b),
                            (tx_a, mx_a), (tx_b, mx_b))):
                        nc.tensor.matmul(tp, tmat[:, sl], mv,
                                         start=(i == 0), stop=(i == 3))
                    nc.scalar.activation(tg[half], tp, AF.Sigmoid, bias=cb72,
                                         scale=GS)
                for half in range(2):
                    nc.sync.dma_start(
                        out=scr_d.rearrange("b n -> n b")[
                            half * MC:(half + 1) * MC, :],
                        in_=tg[half])
                trow = sa_pool.tile([1, NT], F16, tag="trow", name="trow")
                nc.sync.dma_start(
                    out=trow,
                    in_=scr_d.rearrange("b n -> (b n)").unsqueeze(0))
                tbc = sa_pool.tile([128, NT], F16, tag="tbc", name="tbc")
                nc.gpsimd.partition_broadcast(tbc, trow, channels=128)

            qkv_gemm(qkv_ps, "k", yt8, ky6, rot="av")

            # SA part B: gate qy
            for c in range(CH):
                nc.vector.scalar_tensor_tensor(
                    qy6[c], tbc, 1.0, qy6[c], op0=ALU.add, op1=ALU.mult)

            # x column-sums (host) for SE mean path
            xsum = [se_pool.tile([128, 16], F16, tag=f"xs{kp}",
                                 name=f"xs{kp}") for kp in range(3)]
            for kp in range(3):
                nc.sync.dma_start(
                    out=xsum[kp].rearrange("p (i b) -> p i b", i=2),
                    in_=xsum_d[kp * 256:(kp + 1) * 256, :].rearrange(
                        "(i p) b -> p i b", i=2))

        # ------------------------------------------------ attention + proj
        proj_sched = {b: [b] for b in range(BC)}
        proj_sched[7] = [7, 8]

        with tc.tile_pool(name="aps", bufs=2, space="PSUM") as s_ps, \
             tc.tile_pool(name="avp", bufs=2, space="PSUM") as av_ps, \
             tc.tile_pool(name="ztps", bufs=1, space="PSUM") as zt_ps, \
             tc.tile_pool(name="seps", bufs=1, space="PSUM") as se_ps, \
             tc.tile_pool(name="pps", bufs=2, space="PSUM") as p_ps, \
             tc.tile_pool(name="es", bufs=2) as es_pool, \
             tc.tile_pool(name="zt", bufs=3) as zt_pool, \
             tc.tile_pool(name="nrm", bufs=2) as nrm_pool, \
             tc.tile_pool(name="ostg", bufs=4) as ostg_pool:

            def v_ops():
                """V GEMM t-chunks + per-batch repacks, emission-ordered so
                repack(b) lands before av(b) is woven (block b+1)."""

                def vchunk(t, half, ci):
                    def op():
                        sx, sw = CHAINS[ci]
                        if ci == 0:
                            if half == 0:
                                ones = vstage[t].rearrange(
                                    "p (h o) -> p h o", o=65)[:, :, 64:65]
                                nc.vector.memset(ones, 1.0)
                            vps[0] = p_ps.tile([128, NF], F32, tag="pp",
                                               name="pp")
                        wv = w8[("v", sw)].rearrange("p (kc d) -> p kc d",
                                                     kc=CH)
                        for kp in range(3):
                            stat = xt8[sx][kp].rearrange(
                                "p (i n) -> p i n", i=2)[
                                :, :, t * 128:(t + 1) * 128]
                            nc.tensor.matmul(
                                vps[0], stat,
                                wv[:, 2 * kp:2 * kp + 2,
                                   half * NF:(half + 1) * NF],
                                start=(ci == 0 and kp == 0),
                                stop=(ci == 2 and kp == 2), perf_mode=DR)
                        if ci == 2:
                            dst3 = vstage[t].rearrange(
                                "p (h o) -> p h o", o=65)[
                                :, half * 6:(half + 1) * 6, 0:64]
                            evict(dst3,
                                  vps[0].rearrange("p (h d) -> p h d", d=64),
                                  rot="v")
                    return op

                vps = [None]

                def repack(b):
                    def op():
                        for j in range(2):
                            row0 = b * N + j * MC
                            pos = 0
                            while pos < MC:
                                t = (row0 + pos) // 128
                                r0 = (row0 + pos) % 128
                                cnt = min(128 - r0, MC - pos)
                                nc.sync.dma_start(
                                    out=v16[b][pos:pos + cnt,
                                               j * 780:(j + 1) * 780],
                                    in_=vstage[t][r0:r0 + cnt, :])
                                pos += cnt
                    return op

                nextb = 0
                for t in range(9):
                    for half in range(2):
                        for ci in range(3):
                            yield 240, vchunk(t, half, ci)
                    while nextb < BC and (nextb + 1) * N <= (t + 1) * 128:
                        yield 0, repack(nextb)
                        nextb += 1
                while nextb < BC:
                    yield 0, repack(nextb)
                    nextb += 1

            def proj_ops(t, zpairs, pairs, erot="v"):
                """(pe_cost_ns, thunk) pieces for one proj t-chunk.
                zpairs: {s: [3 x [128, 2, NT] chunk-pair views]}"""
                for wph, wpl, od in pairs:
                    wvs = {"h": wph.rearrange("p (kc d) -> p kc d", kc=CH),
                           "l": wpl.rearrange("p (kc d) -> p kc d", kc=CH)}
                    stage = ostg_pool.tile([128, DIM], F16, tag="ostg",
                                           name="ostg")
                    for nf in range(2):
                        ps = p_ps.tile([128, NF], F32, tag="pp", name="pp")
                        for ci, (sz, sw) in enumerate(CHAINS):
                            for kc2 in range(3):
                                def mm(ps=ps, nf=nf, ci=ci, sz=sz, sw=sw,
                                       kc2=kc2):
                                    lw = zpairs[sz][kc2][
                                        :, :, t * 128:(t + 1) * 128]
                                    nc.tensor.matmul(
                                        ps, lw,
                                        wvs[sw][:, 2 * kc2:2 * kc2 + 2,
                                                nf * NF:(nf + 1) * NF],
                                        start=(ci == 0 and kc2 == 0),
                                        stop=(ci == 2 and kc2 == 2),
                                        perf_mode=DR)
                                yield 80, mm
                        def ev(ps=ps, nf=nf, stage=stage):
                            dst = stage[:, nf * NF:(nf + 1) * NF]
                            evict(dst, ps, rot=erot)
                        yield 0, ev
                    def dma(od=od, stage=stage):
                        nc.sync.dma_start(out=od[t * 128:(t + 1) * 128, :],
                                          in_=stage)
                    yield 0, dma

            def qkv_ops(wname, src, dst6, rot):
                for m in range(CH):
                    for nf in range(3):
                        ps = p_ps.tile([128, NF], F32, tag="pp", name="pp")
                        for ci in range(3):
                            def mmc(ps=ps, m=m, nf=nf, ci=ci):
                                sx, sw = CHAINS[ci]
                                wv = w8[(wname, sw)].rearrange(
                                    "p (kc d) -> p kc d", kc=CH)
                                for kp in range(3):
                                    mov = src[sx][kp].rearrange(
                                        "p (i n) -> p i n", i=2)[
                                        :, :, nf * NF:(nf + 1) * NF]
                                    nc.tensor.matmul(
                                        ps, wv[:, 2 * kp:2 * kp + 2,
                                               m * 128:(m + 1) * 128],
                                        mov, start=(ci == 0 and kp == 0),
                                        stop=(ci == 2 and kp == 2),
                                        perf_mode=DR)
                                if ci == 2:
                                    evict(dst6[m][:, nf * NF:(nf + 1) * NF],
                                          ps, rot)
                            yield 240, mmc

            se_state = {}

            def se_ops():
                """SE gate chain as weave pieces (needs all qx6 written)"""
                sw1t = se_pool.tile([128, CH * HID], F16, tag="sw1", name="sw1")
                sw1qt = se_pool.tile([128, CH * HID], F16, tag="sw1q",
                                     name="sw1q")
                sw2t = se_pool.tile([HID, DIM], F16, tag="sw2", name="sw2")
                maxs = [se_pool.tile([128, BC], F16, tag=f"max{c}",
                                     name=f"max{c}") for c in range(CH)]

                def ld():
                    nc.sync.dma_start(
                        out=sw1t.rearrange("p (kc h) -> p kc h", kc=CH),
                        in_=sw1_d.rearrange("(kc p) h -> p kc h", p=128))
                    nc.sync.dma_start(
                        out=sw1qt.rearrange("p (kc h) -> p kc h", kc=CH),
                        in_=sw1q_d.rearrange("(kc p) h -> p kc h", p=128))
                    nc.sync.dma_start(out=sw2t, in_=sw2_d[:, :])
                yield 0, ld
                for c in range(CH):
                    def red(c=c):
                        q3 = qx6[c].rearrange("p (b n) -> p b n", n=N)
                        with nc.allow_low_precision(reason="SE pooling"):
                            nc.vector.reduce_max(maxs[c], q3, axis=AX.X)
                    yield 0, red
                hids = []
                def fc1m():
                    # 1/N mean-scale host-folded into sw1q; relu on DVE so
                    # ACT keeps its Exp table loaded through this region
                    ps = se_ps.tile([128, NF], F32, tag="sep", name="sep")
                    fc1p = ps[0:HID, 0:BC]
                    sw1qv = sw1qt.rearrange("p (kc h) -> p kc h", kc=CH)
                    first = True
                    for kp in range(3):
                        xs3 = xsum[kp].rearrange("p (i b) -> p i b", i=2)
                        for i in range(2):
                            nc.tensor.matmul(fc1p, sw1qv[:, kp * 2 + i, :],
                                             xs3[:, i, :], start=first,
                                             stop=(kp == 2 and i == 1))
                            first = False
                    hid = se_pool.tile([HID, BC], F16, tag="hidm",
                                       name="hidm")
                    nc.vector.tensor_scalar_max(hid, fc1p, 0.0)
                    hids.append(hid)
                yield 100, fc1m
                def fc1x():
                    ps = se_ps.tile([128, NF], F32, tag="sep", name="sep")
                    fc1p = ps[0:HID, 0:BC]
                    sw1v = sw1t.rearrange("p (kc h) -> p kc h", kc=CH)
                    for c in range(CH):
                        nc.tensor.matmul(fc1p, sw1v[:, c, :], maxs[c],
                                         start=(c == 0), stop=(c == CH - 1))
                    hid = se_pool.tile([HID, BC], F16, tag="hidx",
                                       name="hidx")
                    nc.vector.tensor_scalar_max(hid, fc1p, 0.0)
                    hids.append(hid)
                yield 100, fc1x
                sgall = se_pool.tile([128, 2 * CH * BC], F16, tag="sgall",
                                     name="sgall")
                se_state["sgall"] = sgall
                def fc2mm():
                    # all 12 fc2 GEMMs -> ONE sigmoid (one ACT table swap
                    # pair instead of 12 amid the exp chain)
                    ps = se_ps.tile([128, NF], F32, tag="sep", name="sep")
                    for c in range(CH):
                        for pi in range(2):
                            sl = slice((c * 2 + pi) * BC,
                                       (c * 2 + pi + 1) * BC)
                            nc.tensor.matmul(ps[:, sl],
                                             sw2t[:, c * 128:(c + 1) * 128],
                                             hids[pi], start=True, stop=True)
                    nc.scalar.activation(sgall, ps[:, 0:2 * CH * BC],
                                         AF.Sigmoid, scale=GS)
                yield 100, fc2mm

            def se_gates():
                """gate-apply pieces, woven into the kx GEMM stretch so the
                gpsimd work is spread out before x blocks need gated qx"""
                for c in range(CH):
                    def fc2(c=c):
                        sgall = se_state["sgall"]
                        g1 = se_pool.tile([128, BC], F16, tag=f"g1{c}",
                                          name=f"g1{c}")
                        nc.vector.scalar_tensor_tensor(
                            g1, sgall[:, 2 * c * BC:(2 * c + 1) * BC], 1.0,
                            sgall[:, (2 * c + 1) * BC:(2 * c + 2) * BC],
                            op0=ALU.add, op1=ALU.add)
                        q3 = qx6[c].rearrange("p (b n) -> p b n", n=N)
                        g3 = g1.unsqueeze(2).to_broadcast((128, BC, N))
                        nc.vector.tensor_tensor(q3, q3, g3, op=ALU.mult)
                    yield 20, fc2

            def weave(gen_a, gen_b, ratio):
                """yield `ratio` pieces of a per piece of b until both end"""
                a, b = iter(gen_a), iter(gen_b)
                while True:
                    done = True
                    for _ in range(ratio):
                        x = next(a, None)
                        if x is not None:
                            done = False
                            yield x
                    x = next(b, None)
                    if x is not None:
                        done = False
                        yield x
                    if done:
                        return

            def av_ops(b, e16, ztgroups):
                """av + normalize + zT for one finished block"""
                col0 = b * N
                for i in range(2):
                    def grp(i=i):
                        zt = zt_pool.tile([MC, DIM], F16, tag="zt", name="zt")
                        rec = nrm_pool.tile([MC, HEADS], F32, tag="rec",
                                            name="rec")
                        for half in range(2):
                            oaug = av_ps.tile([MC, 6 * 65], F32, tag="oa",
                                              name="oa")
                            for hh in range(6):
                                h = half * 6 + hh
                                for j in range(2):
                                    lhs = e16[h][:, j * N + i * MC:
                                                 j * N + (i + 1) * MC]
                                    rhs = v16[b][:, j * 780 + h * 65:
                                                 j * 780 + (h + 1) * 65]
                                    nc.tensor.matmul(
                                        oaug[:, hh * 65:(hh + 1) * 65],
                                        lhs, rhs, start=(j == 0),
                                        stop=(j == 1))
                            o3 = oaug.rearrange("p (h o) -> p h o", o=65)
                            rsl = rec[:, half * 6:(half + 1) * 6]
                            nc.vector.reciprocal(rsl, o3[:, :, 64:65])
                            z3 = zt.rearrange("p (h d) -> p h d", d=64)[
                                :, half * 6:(half + 1) * 6, :]
                            r3 = rsl.unsqueeze(2).to_broadcast((MC, 6, 64))
                            nc.vector.tensor_tensor(z3, o3[:, :, 0:64],
                                                    r3, op=ALU.mult)
                        ztp = zt_ps.tile([128, CH * MC], F16, tag="ztp",
                                         name="ztp")
                        for c in range(CH):
                            nc.tensor.transpose(ztp[:, c * MC:(c + 1) * MC],
                                                zt[:, c * 128:(c + 1) * 128],
                                                id16[0:MC, 0:MC])
                        z3p = ztp.rearrange("p (c n) -> p c n", c=CH)
                        ecol = col0 + i * MC
                        for view3h, view3l, cnt, c0 in ztgroups:
                            hslc = view3h[:, :, ecol:ecol + MC]
                            evict(hslc, z3p[:, c0:c0 + cnt, :], rot="av")
                            nc.vector.tensor_tensor(
                                view3l[:, :, ecol:ecol + MC],
                                z3p[:, c0:c0 + cnt, :], hslc,
                                op=ALU.subtract)
                    yield 450, grp

            urgent = deque()
            prep = [iter(())]
            bulk = [iter(())]

            def drip(budget):
                while urgent:
                    cost, op = urgent.popleft()
                    op()
                    budget -= max(cost, 20)
                while budget > 0:
                    cost_op = next(prep[0], None)
                    if cost_op is None:
                        cost_op = next(bulk[0], None)
                        if cost_op is None:
                            return
                    cost, op = cost_op
                    op()
                    budget -= max(cost, 20)

            def s_exp_block(b, qq, kk):
                col0 = b * N
                e16 = []
                for h in range(HEADS):
                    c6 = h // 2
                    p0 = (h % 2) * 64
                    q_ap = qq[c6][p0:p0 + 64, col0:col0 + N]
                    sps = s_ps.tile([MC, 2 * N], F32, tag="S", name="S")
                    for j in range(2):
                        k_ap = kk[c6][p0:p0 + 64,
                                      col0 + j * MC:col0 + (j + 1) * MC]
                        nc.tensor.matmul(sps[:, j * N:(j + 1) * N],
                                         k_ap, q_ap, start=True, stop=True)
                    e = es_pool.tile([MC, 2 * N], F16, tag=f"e16_{h}",
                                     name=f"e16_{h}")
                    nc.scalar.activation(e, sps, AF.Exp, scale=ESC)
                    e16.append(e)
                    drip(350)
                return e16

            # merged block order: first x blocks slot into y's ACT tail.
            # proj chunks enter the weave two blocks after their zT rows
            # land, shifting PE filler toward ACT-bound stretches.
            ztg_y = [(zTy8["h"].rearrange("p (c n) -> p c n", c=CH),
                      zTy8["l"].rearrange("p (c n) -> p c n", c=CH), CH, 0)]
            ztg_x = [(xt8["h"][kp].rearrange("p (i n) -> p i n", i=2),
                      xt8["l"][kp].rearrange("p (i n) -> p i n", i=2),
                      2, kp * 2)
                     for kp in range(3)]
            cfg = {"y": (qy6, ky6, "y", "y1", "yo", ztg_y),
                   "x": (qx6, kx6, "x", "x1", "xo", ztg_x)}
            zt_views = {"y": zpy, "x": zpx}
            order = ([("y", b) for b in range(6)]
                     + [("x", 0), ("y", 6), ("x", 1), ("y", 7)]
                     + [("x", b) for b in range(2, BC)])
            lag = {"y": deque(), "x": deque()}

            def queue_proj(sd, pb):
                qq, kk, zkey, o1, o2, _ztg = cfg[sd]
                erot = "vva"
                for t in proj_sched[pb]:
                    bulk[0] = chain(bulk[0], proj_ops(
                        t, zt_views[zkey],
                        ((wp8[(1, "h")], wp8[(1, "l")], outs_d[o1]),
                         (wp8[(2, "h")], wp8[(2, "l")], outs_d[o2])),
                        erot=erot))

            # x-side prep woven into the y region
            prep[0] = chain(v_ops(),
                            qkv_ops("q", xt8, qx6, "va"),
                            se_ops(),
                            weave(qkv_ops("k", xt8, kx6, "va"),
                                  se_gates(), 8))
            prev = None
            for oi, (sd, b) in enumerate(order):
                if prev is not None:
                    psd, pb, pe = prev
                    urgent.extend(av_ops(pb, pe, cfg[psd][5]))
                    lag[psd].append(pb)
                    if len(lag[psd]) >= 2:
                        queue_proj(psd, lag[psd].popleft())
                if oi >= len(order) - 3:
                    # tail: drain lagged projections early so the last
                    # blocks' PE filler doesn't pile up after the exp chain
                    for s2 in ("y", "x"):
                        while lag[s2]:
                            queue_proj(s2, lag[s2].popleft())
                if (sd, b) == ("x", 0):
                    # x blocks read qx/kx/SE outputs; emit any remaining
                    # prep pieces now (normally already drained)
                    for cost, op in prep[0]:
                        op()
                    prep[0] = iter(())
                e16 = s_exp_block(b, *cfg[sd][:2])
                prev = (sd, b, e16)
            psd, pb, pe = prev
            urgent.extend(av_ops(pb, pe, cfg[psd][5]))
            lag[psd].append(pb)
            for sd in ("y", "x"):
                while lag[sd]:
                    queue_proj(sd, lag[sd].popleft())
            while urgent:
                urgent.popleft()[1]()
            for cost, op in bulk[0]:
                op()


def _f8(a):
    return np.clip(a, -240.0, 240.0).astype(ml_dtypes.float8_e4m3)


def _split8(a):
    hi = _f8(a)
    lo = _f8(np.asarray(a, np.float64) - hi.astype(np.float64))
    return hi, lo


def _toeplitz(k5):
    """[144,144] T with T[m,n] = k5[my-ny+2, mx-nx+2]"""
    t = np.zeros((N, N), np.float64)
    for ny in range(PATCH):
        for nx in range(PATCH):
            for dy in range(-2, 3):
                for dx in range(-2, 3):
                    my, mx = ny + dy, nx + dx
                    if 0 <= my < PATCH and 0 <= mx < PATCH:
                        t[my * PATCH + mx, ny * PATCH + nx] = k5[dy + 2, dx + 2]
    return t


def _prep_weights(inputs):
    Wqkv = np.asarray(inputs["Wqkv"], np.float64)
    w = {}
    for i, nm in enumerate("qkv"):
        hi, lo = _split8(WS * Wqkv[:, (i + 1) * DIM:(i + 2) * DIM])
        w[f"w{nm}h"], w[f"w{nm}l"] = hi, lo
    wp64 = np.asarray(inputs["Wproj"], np.float64)
    w["wp1h"], w["wp1l"] = _split8(WS * wp64)
    w["wp2h"], w["wp2l"] = _split8(WS * (wp64 @ wp64))
    bp = np.asarray(inputs["bproj"], np.float64).reshape(1, DIM)
    bp2 = (bp @ wp64 + bp).astype(np.float32)
    sw1_64 = np.asarray(inputs["se_w1"], np.float64)
    w["sw1"] = sw1_64.astype(np.float16)
    wq64 = Wqkv[:, DIM:2 * DIM]
    w["sw1q"] = ((wq64 @ sw1_64) / N).astype(np.float16)
    wqmean = wq64.mean(axis=1)
    w["sw2"] = np.asarray(inputs["se_w2"], np.float16)
    sa_w = np.asarray(inputs["sa_w"], np.float64)
    w["tm"] = _toeplitz(sa_w[0, 0]).astype(np.float16)
    w["tx"] = _toeplitz(sa_w[0, 1]).astype(np.float16)
    w["cb"] = np.asarray(inputs["sa_b"], np.float32).reshape(1, 1)
    return w, bp.astype(np.float32), bp2, wqmean


def kernel(**inputs):
    from concourse.bass_utils import run_bass_kernel_spmd

    if "nc" not in _COMPILED:
        _COMPILED["nc"] = build_program()
    nc = _COMPILED["nc"]

    w, bp, bp2, wqmean = _prep_weights(inputs)
    x = np.asarray(inputs["x"], np.float32).reshape(B, N, DIM)
    y = np.asarray(inputs["y"], np.float32).reshape(B, N, DIM)
    in_maps = []
    for i in range(NCORES):
        m = dict(w)
        xsh = x[i * BC:(i + 1) * BC].reshape(NT, DIM)
        ysh = y[i * BC:(i + 1) * BC].reshape(NT, DIM)
        m["xh"], m["xl"] = _split8(np.ascontiguousarray(xsh.T))
        m["yh"], m["yl"] = _split8(np.ascontiguousarray(ysh.T))
        m["xsum"] = np.ascontiguousarray(
            WS * xsh.reshape(BC, N, DIM).sum(axis=1, dtype=np.float64).T
        ).astype(np.float16)
        m["msd"] = np.ascontiguousarray(
            WS * (ysh.astype(np.float64) @ wqmean).reshape(BC, N).T
        ).astype(np.float16)
        in_maps.append(m)

    res = run_bass_kernel_spmd(nc, in_maps, core_ids=list(range(NCORES)))
    inv = 1.0 / (WS * WS)
    outs = []
    for name, bias in (("x1", bp), ("y1", bp), ("xo", bp2), ("yo", bp2)):
        full = np.concatenate(
            [np.asarray(res.results[i][name], np.float32).reshape(BC, N, DIM)
             for i in range(NCORES)], axis=0)
        outs.append(full * inv + bias.reshape(1, 1, DIM))
    return tuple(outs)


# revision 51
# speedup vs baseline: 1.1509x; 1.0335x over previous
"""Trainium2 Bass kernel for nn_Attention_29326036697657 (sparse_attention).

Dual-input attention with SE (channel) / SA (spatial) gates.
Sharding: data-parallel over batch B=64 across 8 cores (8 batches/core).

Algebraic folds (vs reference):
  - qxo/qyo/attnx dead -> Wqkv comp 0 unused; vy = vx (reference quirk).
  - dots(q,k)+dots(q2,k) = dots(q*(1+g), k) for both SE and SA gates.
  - softmax without max-subtraction (logits O(1)); denominator via a
    ones column appended to each V head block.
  - bias + scale fixups applied on HOST after gather.
  - SA 5x5 conv lowered to two host-built Toeplitz matmuls (TM/768, TX).

Numerics: the five qkv GEMMs and four proj GEMMs run as split-fp8
DoubleRow matmuls (residual decomposition a=ah+al, w=wh+wl, dropping
the al*wl term): 3 chains x 3 DR steps of K=256 each at 0.5 cyc/row
vs fp16's 6 steps at 1.0 -- 25% fewer PE cycles at ~1e-3 accuracy.
Weights are host-scaled x64 so fp8 residuals clear the subnormal
floor; activations carry x64 which folds into the exp scale
(ESC/4096), the two sigmoid scales (1/64), and a host-side /4096.
S and AV GEMMs stay fp16 (single-fp8 q/k measured >2e-2). zT is
split to fp8 hi+lo on eviction (one extra DVE pass per block).

Layout: inputs arrive HOST-pre-quantized ([768, NT] fp8 hi+lo) so the
kernel does zero input transposes/quantization. q/k tiles [128, NT]
f16; v natural per-batch [72, 2*780] f16 with denominator columns; z
transposed back via PE (f16 identity); proj reads fp8 zT slabs, psum
evicted f16 and DMA'd per t-chunk.

Schedule: software-pipelined around the ACT exp chain: attention
S+exp blocks drip a cost-budgeted weave of V/qk-GEMMs, the SE chain,
and projections (2-block lag) into exp-wait gaps; x-side blocks
interleave into the y-side ACT tail. kx aliases the dead y-input fp8
tiles (bitcast), zTx h/l alias the dead x-input fp8 tiles. GPSIMD
cannot touch PSUM, so evictions rotate across DVE/ACT only.
"""

import sys

sys.path.insert(0, "/opt/trn_rl_repo")

import numpy as np
import ml_dtypes

import concourse.bass as bass
import concourse.bacc as bacc_mod
import concourse.mybir as mybir
import concourse.tile as tile
from concourse.masks import make_identity

# ---------------------------------------------------------------- constants
DIM = 768
HEADS = 12
PATCH = 12
N = PATCH * PATCH          # 144
B = 64
RED = 16
HID = DIM // RED           # 48
HD = DIM // HEADS          # 64
SCALE = HD ** -0.5         # 0.125

NCORES = 8
BC = B // NCORES           # 8 batches per core
NT = BC * N                # 1152 rows per core
CH = DIM // 128            # 6 channel chunks
NF = 384                   # qkv/proj moving chunk
MC = 72                    # m/n half within one batch
WS = 64.0                  # host weight scale (fp8 sweet spot)
ESC = SCALE / (WS * WS)    # exp scale (q,k each carry x64)
GS = 1.0 / WS              # gate sigmoid input unscale

F32 = mybir.dt.float32
F16 = mybir.dt.float16
BF16 = mybir.dt.bfloat16
F8 = mybir.dt.float8e4
AX = mybir.AxisListType
AF = mybir.ActivationFunctionType
ALU = mybir.AluOpType
DR = mybir.MatmulPerfMode.DoubleRow

_COMPILED = {}


def build_program():
    nc = bacc_mod.Bacc()

    # packed layouts: per 256-row chunk, hi rows then lo rows (halves the
    # DMA count -- each HWDGE issue costs ~625ns of serial queue time)
    xin_d = nc.dram_tensor("xp", [2 * DIM, NT], F8, kind="ExternalInput")
    yin_d = nc.dram_tensor("yp", [2 * DIM, NT], F8, kind="ExternalInput")
    w8_d = {w: nc.dram_tensor(f"w{w}p", [2 * DIM, DIM], F8,
                              kind="ExternalInput")
            for w in "qkv"}
    wp8_d = {p: nc.dram_tensor(f"wp{p}p", [2 * DIM, DIM], F8,
                               kind="ExternalInput")
             for p in (1, 2)}
    sw1_d = nc.dram_tensor("sw1", [DIM, HID], F16, kind="ExternalInput")
    sw1q_d = nc.dram_tensor("sw1q", [DIM, HID], F16, kind="ExternalInput")
    xsum_d = nc.dram_tensor("xsum", [DIM, BC], F16, kind="ExternalInput")
    msd_d = nc.dram_tensor("msd", [N, BC], F16, kind="ExternalInput")
    sw2_d = nc.dram_tensor("sw2", [HID, DIM], F16, kind="ExternalInput")
    tm_d = nc.dram_tensor("tm", [N, N], F16, kind="ExternalInput")
    tx_d = nc.dram_tensor("tx", [N, N], F16, kind="ExternalInput")
    cb_d = nc.dram_tensor("cb", [1, 1], F32, kind="ExternalInput")
    scr_d = nc.dram_tensor("scr", [BC, N], F16, kind="ExternalOutput")
    outs_d = {
        nm: nc.dram_tensor(nm, [NT, DIM], F16, kind="ExternalOutput")
        for nm in ("x1", "y1", "xo", "yo")
    }

    with tile.TileContext(nc) as tc:
        _body(nc, tc, xin_d, yin_d, w8_d, wp8_d,
              sw1_d, sw1q_d, xsum_d, msd_d, sw2_d, tm_d, tx_d, cb_d, scr_d,
              outs_d)
    nc.compile()
    return nc

def _body(nc, tc, xin_d, yin_d, w8_d, wp8_d,
          sw1_d, sw1q_d, xsum_d, msd_d, sw2_d, tm_d, tx_d, cb_d, scr_d,
          outs_d):
    from contextlib import ExitStack
    from collections import deque
    from itertools import chain

    est = ExitStack()
    with est:
        const = est.enter_context(tc.tile_pool(name="const", bufs=1))
        id16 = const.tile([128, 128], F16, tag="id16", name="id16")
        make_identity(nc, id16)
        idbf = const.tile([128, 128], BF16, tag="idbf", name="idbf")
        make_identity(nc, idbf)
        cb72 = const.tile([MC, 1], F32, tag="cb72", name="cb72")

        # persistent activation tiles
        act = est.enter_context(tc.tile_pool(name="act", bufs=1))
        qx6 = [act.tile([128, NT], F16, tag=f"qx{c}", name=f"qx{c}") for c in range(CH)]
        qy6 = [act.tile([128, NT], F16, tag=f"qy{c}", name=f"qy{c}") for c in range(CH)]
        ky6 = [act.tile([128, NT], F16, tag=f"ky{c}", name=f"ky{c}") for c in range(CH)]
        v16 = [act.tile([MC, 2 * 780], F16, tag=f"v16_{b}", name=f"v16_{b}")
               for b in range(BC)]
        zTy8 = {s: act.tile([128, CH * NT], F8, tag=f"zTy{s}", name=f"zTy{s}")
                for s in "hl"}
        wp8 = {p: act.tile([128, 2 * CH * DIM], F8, tag=f"wp{p}",
                           name=f"wp{p}")
               for p in (1, 2)}
        sa_pool = est.enter_context(tc.tile_pool(name="sa", bufs=1))
        se_pool = est.enter_context(tc.tile_pool(name="se", bufs=1))
        # qkv inputs/weights (persistent: x GEMMs weave into the y region)
        qkw = est.enter_context(tc.tile_pool(name="qkw", bufs=1))
        xt8 = [qkw.tile([128, 4 * NT], F8, tag=f"xt{kp}",
                        name=f"xt{kp}") for kp in range(3)]
        yt8 = [qkw.tile([128, 4 * NT], F8, tag=f"yt{kp}",
                        name=f"yt{kp}") for kp in range(3)]
        w8 = {w: qkw.tile([128, 2 * CH * DIM], F8, tag=f"w{w}",
                          name=f"w{w}")
              for w in "qkv"}
        vstage = [qkw.tile([128, HEADS * 65], F16, tag=f"vs{t}",
                           name=f"vs{t}") for t in range(9)]

        # packed-tile views: tile kp holds [s(hi/lo), i(2 kc), n] groups
        SI = {"h": 0, "l": 1}

        def xin_pair(tiles, s, kp):
            return tiles[kp].rearrange("p (s i n) -> p s i n", s=2, i=2)[
                :, SI[s], :, :]

        def w_pair(tile, s, c2):
            return tile.rearrange("p (c s i d) -> p c s i d", c=3, s=2, i=2)[
                :, c2, SI[s], :, :]

        # aliases: kx (fp16) lives in the dead y fp8 tiles via bitcast;
        # zTx h/l live in the dead x fp8 tiles (same dtype slicing).
        kx6 = [yt8[c // 2].bitcast(F16)[:, (c % 2) * NT:(c % 2 + 1) * NT]
               for c in range(CH)]
        # [128, 2, NT] chunk-pair views for the proj stationary reads
        zpx = {s: [xin_pair(xt8, s, kc2) for kc2 in range(3)] for s in "hl"}
        zpy = {s: [zTy8[s].rearrange("p (c n) -> p c n", c=CH)[
                       :, 2 * kc2:2 * kc2 + 2, :]
                   for kc2 in range(3)] for s in "hl"}

        evrot = [0]

        def evict(dst, src, rot="va"):
            """rotate psum evictions across DVE/ACT"""
            r = rot[evrot[0] % len(rot)]
            evrot[0] += 1
            if r == "a":
                nc.scalar.copy(dst, src)
            else:
                nc.vector.tensor_copy(dst, src)

        def load_w_c2(tile, w_d, c2):
            # one DMA per kc-pair covering hi+lo rows (packed dram layout)
            nc.sync.dma_start(
                out=tile.rearrange("p (c g d) -> p c g d", c=3, g=4)[
                    :, c2, :, :],
                in_=w_d[c2 * 512:(c2 + 1) * 512, :].rearrange(
                    "(g p) d -> p g d", p=128))

        def load_w(tile, w_d):
            for c2 in range(3):
                load_w_c2(tile, w_d, c2)

        def load_in_kp(dst, src_d, kp):
            nc.sync.dma_start(
                out=dst[kp].rearrange("p (g n) -> p g n", g=4),
                in_=src_d[kp * 512:(kp + 1) * 512, :].rearrange(
                    "(g p) n -> p g n", p=128))

        def load_in_half(dst, src_d, kp, si):
            # half-chunk (hi or lo) load: finer startup pipelining
            nc.sync.dma_start(
                out=dst[kp].rearrange("p (s i n) -> p s i n", s=2, i=2)[
                    :, si, :, :],
                in_=src_d[kp * 512 + si * 256:kp * 512 + (si + 1) * 256,
                          :].rearrange("(i p) n -> p i n", p=128))

        # load order = first-use order, wq/y interleaved per chunk with
        # hi halves first (the hh chain runs on them alone)
        for c2 in range(3):
            load_w_c2(w8["q"], w8_d["q"], c2)
            load_in_half(yt8, yin_d, c2, 0)
        for c2 in range(3):
            load_in_half(yt8, yin_d, c2, 1)
        load_w(w8["k"], w8_d["k"])
        nc.sync.dma_start(out=cb72, in_=cb_d[:, :].to_broadcast((MC, 1)))
        # small SA-gate inputs next: they gate the attention start, and
        # behind the big x/wv/wp transfers they'd starve the SA chain
        ms_a = sa_pool.tile([128, BC], F16, tag="msa", name="msa")
        ms_b = sa_pool.tile([16, BC], F16, tag="msb", name="msb")
        tm_a = sa_pool.tile([128, N], F16, tag="tma", name="tma")
        tm_b = sa_pool.tile([16, N], F16, tag="tmb", name="tmb")
        tx_a = sa_pool.tile([128, N], F16, tag="txa", name="txa")
        tx_b = sa_pool.tile([16, N], F16, tag="txb", name="txb")
        nc.sync.dma_start(out=ms_a, in_=msd_d[0:128, :])
        nc.sync.dma_start(out=ms_b, in_=msd_d[128:144, :])
        nc.sync.dma_start(out=tm_a, in_=tm_d[0:128, :])
        nc.sync.dma_start(out=tm_b, in_=tm_d[128:144, :])
        nc.sync.dma_start(out=tx_a, in_=tx_d[0:128, :])
        nc.sync.dma_start(out=tx_b, in_=tx_d[128:144, :])
        for kp in range(3):
            load_in_kp(xt8, xin_d, kp)
        load_w(w8["v"], w8_d["v"])
        # wp loads go last -- first used by proj (~60us in)
        for p in (1, 2):
            load_w(wp8[p], wp8_d[p])

        CHAINS = (("h", "h"), ("h", "l"), ("l", "h"))

        def qkv_chunk(psum, wname, src, dst6, m, nf, rot, tag="qkv"):
            ps = psum.tile([128, NF], F32, tag=tag, name=tag)
            for ci, (sx, sw) in enumerate(CHAINS):
                for kp in range(3):
                    mov = xin_pair(src, sx, kp)[:, :, nf * NF:(nf + 1) * NF]
                    lw = w_pair(w8[wname], sw, kp)[:, :,
                                                   m * 128:(m + 1) * 128]
                    nc.tensor.matmul(
                        ps, lw, mov, start=(ci == 0 and kp == 0),
                        stop=(ci == 2 and kp == 2), perf_mode=DR)
            evict(dst6[m][:, nf * NF:(nf + 1) * NF], ps, rot)

        def qkv_gemm(psum, wname, src, dst6, rot="va", after_m=None):
            for m in range(CH):
                for nf in range(3):
                    qkv_chunk(psum, wname, src, dst6, m, nf, rot)
                if after_m is not None:
                    after_m(m)

        satp_box = [None]
        tbc = sa_pool.tile([128, NT], F16, tag="tbc", name="tbc")

        def sa_partA():
            # SA gate: accm max (DVE), transpose-pool (PE), toeplitz conv +
            # sigmoid, scr roundtrip, broadcast. Emitted a few ky chunks in
            # so the PE queue reaches the transposes after accm is done.
            accm = sa_pool.tile([128, NT], BF16, tag="accm", name="accm")
            nc.vector.tensor_max(accm, qy6[0], qy6[1])
            for c in range(2, CH):
                nc.vector.tensor_max(accm, accm, qy6[c])
            mx_a = sa_pool.tile([128, BC], F16, tag="mxa", name="mxa")
            mx_b = sa_pool.tile([16, BC], F16, tag="mxb", name="mxb")
            satp = satp_box[0]
            pa = satp.tile([128, BC * 128], BF16, tag="pa", name="pa")
            pb = satp.tile([16, BC * 128], BF16, tag="pb", name="pb")
            for b in range(BC):
                nc.tensor.transpose(
                    pa[:, b * 128:(b + 1) * 128],
                    accm[:, b * N:b * N + 128], idbf)
                nc.tensor.transpose(
                    pb[:, b * 128:(b + 1) * 128],
                    accm[:, b * N + 128:(b + 1) * N], idbf)
            with nc.allow_low_precision(reason="SA gate pooling"):
                nc.vector.reduce_max(
                    mx_a, pa.rearrange("p (b n) -> p b n", n=128), axis=AX.X)
                nc.vector.reduce_max(
                    mx_b, pb.rearrange("p (b n) -> p b n", n=128), axis=AX.X)
            # toeplitz conv + sigmoid gate (tm/tx/ms loaded early)
            tg = [sa_pool.tile([MC, BC], F16, tag=f"tg{h}", name=f"tg{h}")
                  for h in range(2)]
            for half in range(2):
                tp = satp.tile([MC, BC], F32, tag="tp", name="tp")
                sl = slice(half * MC, (half + 1) * MC)
                for i, (tmat, mv) in enumerate((
                        (tm_a, ms_a), (tm_b, ms_b),
                        (tx_a, mx_a), (tx_b, mx_b))):
                    nc.tensor.matmul(tp, tmat[:, sl], mv,
                                     start=(i == 0), stop=(i == 3))
                nc.scalar.activation(tg[half], tp, AF.Sigmoid, bias=cb72,
                                     scale=GS)
            for half in range(2):
                nc.sync.dma_start(
                    out=scr_d.rearrange("b n -> n b")[
                        half * MC:(half + 1) * MC, :],
                    in_=tg[half])
            trow = sa_pool.tile([1, NT], F16, tag="trow", name="trow")
            nc.sync.dma_start(
                out=trow,
                in_=scr_d.rearrange("b n -> (b n)").unsqueeze(0))
            nc.gpsimd.partition_broadcast(tbc, trow, channels=128)

        with tc.tile_pool(name="qkvps", bufs=4, space="PSUM") as qkv_ps, \
             tc.tile_pool(name="satp", bufs=1, space="PSUM") as satp:
            satp_box[0] = satp
            qkv_gemm(qkv_ps, "q", yt8, qy6)

            def gate_qy(c):
                nc.vector.scalar_tensor_tensor(
                    qy6[c], tbc, 1.0, qy6[c], op0=ALU.add, op1=ALU.mult)

            SA_AT, GATE_FROM = 0, 3

            def k_hook(m):
                if m == SA_AT:
                    sa_partA()
                if m >= GATE_FROM:
                    gate_qy(m - GATE_FROM)

            qkv_gemm(qkv_ps, "k", yt8, ky6, rot="av", after_m=k_hook)
            for c in range(CH - GATE_FROM, CH):
                gate_qy(c)

            # x column-sums (host) for SE mean path
            xsum = [se_pool.tile([128, 16], F16, tag=f"xs{kp}",
                                 name=f"xs{kp}") for kp in range(3)]
            for kp in range(3):
                nc.sync.dma_start(
                    out=xsum[kp].rearrange("p (i b) -> p i b", i=2),
                    in_=xsum_d[kp * 256:(kp + 1) * 256, :].rearrange(
                        "(i p) b -> p i b", i=2))

        # ------------------------------------------------ attention + proj
        proj_sched = {b: [b] for b in range(BC)}
        proj_sched[7] = [7, 8]

        with tc.tile_pool(name="aps", bufs=2, space="PSUM") as s_ps, \
             tc.tile_pool(name="avp", bufs=2, space="PSUM") as av_ps, \
             tc.tile_pool(name="ztps", bufs=1, space="PSUM") as zt_ps, \
             tc.tile_pool(name="pps", bufs=3, space="PSUM") as p_ps, \
             tc.tile_pool(name="es", bufs=2) as es_pool, \
             tc.tile_pool(name="zt", bufs=3) as zt_pool, \
             tc.tile_pool(name="nrm", bufs=2) as nrm_pool, \
             tc.tile_pool(name="ostg", bufs=4) as ostg_pool:

            def v_ops():
                """V GEMM t-chunks + per-batch repacks, emission-ordered so
                repack(b) lands before av(b) is woven (block b+1)."""

                def vchunk(t, half, ci):
                    def op():
                        sx, sw = CHAINS[ci]
                        if ci == 0:
                            if half == 0:
                                ones = vstage[t].rearrange(
                                    "p (h o) -> p h o", o=65)[:, :, 64:65]
                                nc.vector.memset(ones, 1.0)
                            vps[0] = p_ps.tile([128, NF], F32, tag="pp",
                                               name="pp")
                        for kp in range(3):
                            stat = xin_pair(xt8, sx, kp)[
                                :, :, t * 128:(t + 1) * 128]
                            mov = w_pair(w8["v"], sw, kp)[
                                :, :, half * NF:(half + 1) * NF]
                            nc.tensor.matmul(
                                vps[0], stat, mov,
                                start=(ci == 0 and kp == 0),
                                stop=(ci == 2 and kp == 2), perf_mode=DR)
                        if ci == 2:
                            dst3 = vstage[t].rearrange(
                                "p (h o) -> p h o", o=65)[
                                :, half * 6:(half + 1) * 6, 0:64]
                            evict(dst3,
                                  vps[0].rearrange("p (h d) -> p h d", d=64),
                                  rot="v")
                    return op

                vps = [None]

                def repack(b):
                    def op():
                        for j in range(2):
                            row0 = b * N + j * MC
                            pos = 0
                            while pos < MC:
                                t = (row0 + pos) // 128
                                r0 = (row0 + pos) % 128
                                cnt = min(128 - r0, MC - pos)
                                nc.sync.dma_start(
                                    out=v16[b][pos:pos + cnt,
                                               j * 780:(j + 1) * 780],
                                    in_=vstage[t][r0:r0 + cnt, :])
                                pos += cnt
                    return op

                nextb = 0
                for t in range(9):
                    for half in range(2):
                        for ci in range(3):
                            yield 240, vchunk(t, half, ci)
                    while nextb < BC and (nextb + 1) * N <= (t + 1) * 128:
                        yield 0, repack(nextb)
                        nextb += 1
                while nextb < BC:
                    yield 0, repack(nextb)
                    nextb += 1

            def proj_ops(t, zpairs, pairs, erot="v", split_dma=False):
                """(pe_cost_ns, thunk) pieces for one proj t-chunk.
                zpairs: {s: [3 x [128, 2, NT] chunk-pair views]}"""
                for wpt, od in pairs:
                    stage = ostg_pool.tile([128, DIM], F16, tag="ostg",
                                           name="ostg")
                    for nf in range(2):
                        ps = p_ps.tile([128, NF], F32, tag="pp", name="pp")
                        for ci, (sz, sw) in enumerate(CHAINS):
                            for kc2 in range(3):
                                def mm(ps=ps, nf=nf, ci=ci, sz=sz, sw=sw,
                                       kc2=kc2):
                                    lw = zpairs[sz][kc2][
                                        :, :, t * 128:(t + 1) * 128]
                                    mov = w_pair(wpt, sw, kc2)[
                                        :, :, nf * NF:(nf + 1) * NF]
                                    nc.tensor.matmul(
                                        ps, lw, mov,
                                        start=(ci == 0 and kc2 == 0),
                                        stop=(ci == 2 and kc2 == 2),
                                        perf_mode=DR)
                                yield 80, mm
                        def ev(ps=ps, nf=nf, stage=stage):
                            dst = stage[:, nf * NF:(nf + 1) * NF]
                            evict(dst, ps, rot=erot)
                            if split_dma:
                                nc.sync.dma_start(
                                    out=od[t * 128:(t + 1) * 128,
                                           nf * NF:(nf + 1) * NF],
                                    in_=dst)
                        yield 0, ev
                    if not split_dma:
                        def dma(od=od, stage=stage):
                            nc.sync.dma_start(
                                out=od[t * 128:(t + 1) * 128, :], in_=stage)
                        yield 0, dma

            def qkv_ops(wname, src, dst6, rot):
                for m in range(CH):
                    for nf in range(3):
                        ps = p_ps.tile([128, NF], F32, tag="pp", name="pp")
                        for ci in range(3):
                            def mmc(ps=ps, m=m, nf=nf, ci=ci):
                                sx, sw = CHAINS[ci]
                                for kp in range(3):
                                    mov = xin_pair(src, sx, kp)[
                                        :, :, nf * NF:(nf + 1) * NF]
                                    lw = w_pair(w8[wname], sw, kp)[
                                        :, :, m * 128:(m + 1) * 128]
                                    nc.tensor.matmul(
                                        ps, lw, mov,
                                        start=(ci == 0 and kp == 0),
                                        stop=(ci == 2 and kp == 2),
                                        perf_mode=DR)
                                if ci == 2:
                                    evict(dst6[m][:, nf * NF:(nf + 1) * NF],
                                          ps, rot)
                            yield 240, mmc

            se_state = {}

            def se_ops():
                """SE gate chain as weave pieces (needs all qx6 written)"""
                sw1t = se_pool.tile([128, CH * HID], F16, tag="sw1", name="sw1")
                sw1qt = se_pool.tile([128, CH * HID], F16, tag="sw1q",
                                     name="sw1q")
                sw2t = se_pool.tile([HID, DIM], F16, tag="sw2", name="sw2")
                maxs = [se_pool.tile([128, BC], F16, tag=f"max{c}",
                                     name=f"max{c}") for c in range(CH)]

                def ld():
                    nc.sync.dma_start(
                        out=sw1t.rearrange("p (kc h) -> p kc h", kc=CH),
                        in_=sw1_d.rearrange("(kc p) h -> p kc h", p=128))
                    nc.sync.dma_start(
                        out=sw1qt.rearrange("p (kc h) -> p kc h", kc=CH),
                        in_=sw1q_d.rearrange("(kc p) h -> p kc h", p=128))
                    nc.sync.dma_start(out=sw2t, in_=sw2_d[:, :])
                yield 0, ld
                for c in range(CH):
                    def red(c=c):
                        q3 = qx6[c].rearrange("p (b n) -> p b n", n=N)
                        with nc.allow_low_precision(reason="SE pooling"):
                            nc.vector.reduce_max(maxs[c], q3, axis=AX.X)
                    yield 0, red
                hids = []
                def fc1m():
                    # 1/N mean-scale host-folded into sw1q; relu on DVE so
                    # ACT keeps its Exp table loaded through this region
                    ps = p_ps.tile([128, NF], F32, tag="pp", name="pp")
                    fc1p = ps[0:HID, 0:BC]
                    sw1qv = sw1qt.rearrange("p (kc h) -> p kc h", kc=CH)
                    first = True
                    for kp in range(3):
                        xs3 = xsum[kp].rearrange("p (i b) -> p i b", i=2)
                        for i in range(2):
                            nc.tensor.matmul(fc1p, sw1qv[:, kp * 2 + i, :],
                                             xs3[:, i, :], start=first,
                                             stop=(kp == 2 and i == 1))
                            first = False
                    hid = se_pool.tile([HID, BC], F16, tag="hidm",
                                       name="hidm")
                    nc.vector.tensor_scalar_max(hid, fc1p, 0.0)
                    hids.append(hid)
                yield 100, fc1m
                def fc1x():
                    ps = p_ps.tile([128, NF], F32, tag="pp", name="pp")
                    fc1p = ps[0:HID, 0:BC]
                    sw1v = sw1t.rearrange("p (kc h) -> p kc h", kc=CH)
                    for c in range(CH):
                        nc.tensor.matmul(fc1p, sw1v[:, c, :], maxs[c],
                                         start=(c == 0), stop=(c == CH - 1))
                    hid = se_pool.tile([HID, BC], F16, tag="hidx",
                                       name="hidx")
                    nc.vector.tensor_scalar_max(hid, fc1p, 0.0)
                    hids.append(hid)
                yield 100, fc1x
                sgall = se_pool.tile([128, 2 * CH * BC], F16, tag="sgall",
                                     name="sgall")
                se_state["sgall"] = sgall
                def fc2mm():
                    # all 12 fc2 GEMMs -> ONE sigmoid (one ACT table swap
                    # pair instead of 12 amid the exp chain)
                    ps = p_ps.tile([128, NF], F32, tag="pp", name="pp")
                    for c in range(CH):
                        for pi in range(2):
                            sl = slice((c * 2 + pi) * BC,
                                       (c * 2 + pi + 1) * BC)
                            nc.tensor.matmul(ps[:, sl],
                                             sw2t[:, c * 128:(c + 1) * 128],
                                             hids[pi], start=True, stop=True)
                    nc.scalar.activation(sgall, ps[:, 0:2 * CH * BC],
                                         AF.Sigmoid, scale=GS)
                yield 100, fc2mm

            def se_gates():
                """gate-apply pieces, woven into the kx GEMM stretch so the
                gpsimd work is spread out before x blocks need gated qx"""
                for c in range(CH):
                    def fc2(c=c):
                        sgall = se_state["sgall"]
                        g1 = se_pool.tile([128, BC], F16, tag=f"g1{c}",
                                          name=f"g1{c}")
                        nc.vector.scalar_tensor_tensor(
                            g1, sgall[:, 2 * c * BC:(2 * c + 1) * BC], 1.0,
                            sgall[:, (2 * c + 1) * BC:(2 * c + 2) * BC],
                            op0=ALU.add, op1=ALU.add)
                        q3 = qx6[c].rearrange("p (b n) -> p b n", n=N)
                        g3 = g1.unsqueeze(2).to_broadcast((128, BC, N))
                        nc.vector.tensor_tensor(q3, q3, g3, op=ALU.mult)
                    yield 20, fc2

            def weave(gen_a, gen_b, ratio):
                """yield `ratio` pieces of a per piece of b until both end"""
                a, b = iter(gen_a), iter(gen_b)
                while True:
                    done = True
                    for _ in range(ratio):
                        x = next(a, None)
                        if x is not None:
                            done = False
                            yield x
                    x = next(b, None)
                    if x is not None:
                        done = False
                        yield x
                    if done:
                        return

            def av_ops(b, e16, ztgroups):
                """av + normalize + zT for one finished block. Split into
                matmul pieces and transpose pieces (mm0, mm1, zt0, zt1) so
                the DVE normalize latency of group i hides under group
                i+1's PE matmuls instead of stalling the PE mid-piece."""
                col0 = b * N
                st = {}

                def mm(i):
                    def op():
                        zt = zt_pool.tile([MC, DIM], F16, tag="zt", name="zt")
                        rec = nrm_pool.tile([MC, HEADS], F32, tag="rec",
                                            name="rec")
                        st[i] = zt
                        for half in range(2):
                            oaug = av_ps.tile([MC, 6 * 65], F32, tag="oa",
                                              name="oa")
                            for hh in range(6):
                                h = half * 6 + hh
                                for j in range(2):
                                    lhs = e16[h][:, j * N + i * MC:
                                                 j * N + (i + 1) * MC]
                                    rhs = v16[b][:, j * 780 + h * 65:
                                                 j * 780 + (h + 1) * 65]
                                    nc.tensor.matmul(
                                        oaug[:, hh * 65:(hh + 1) * 65],
                                        lhs, rhs, start=(j == 0),
                                        stop=(j == 1))
                            o3 = oaug.rearrange("p (h o) -> p h o", o=65)
                            rsl = rec[:, half * 6:(half + 1) * 6]
                            nc.vector.reciprocal(rsl, o3[:, :, 64:65])
                            z3 = zt.rearrange("p (h d) -> p h d", d=64)[
                                :, half * 6:(half + 1) * 6, :]
                            r3 = rsl.unsqueeze(2).to_broadcast((MC, 6, 64))
                            nc.vector.tensor_tensor(z3, o3[:, :, 0:64],
                                                    r3, op=ALU.mult)
                    return op

                def ztf(i):
                    def op():
                        zt = st.pop(i)
                        ztp = zt_ps.tile([128, CH * MC], F16, tag="ztp",
                                         name="ztp")
                        for c in range(CH):
                            nc.tensor.transpose(ztp[:, c * MC:(c + 1) * MC],
                                                zt[:, c * 128:(c + 1) * 128],
                                                id16[0:MC, 0:MC])
                        z3p = ztp.rearrange("p (c n) -> p c n", c=CH)
                        ecol = col0 + i * MC
                        for view3h, view3l, cnt, c0 in ztgroups:
                            hslc = view3h[:, :, ecol:ecol + MC]
                            evict(hslc, z3p[:, c0:c0 + cnt, :], rot="av")
                            nc.vector.tensor_tensor(
                                view3l[:, :, ecol:ecol + MC],
                                z3p[:, c0:c0 + cnt, :], hslc,
                                op=ALU.subtract)
                    return op

                yield 500, mm(0)
                yield 500, mm(1)
                yield 200, ztf(0)
                yield 200, ztf(1)

            urgent = deque()
            prep = [iter(())]
            bulk = [iter(())]

            def drip(budget):
                # one priority (av) piece per drip: spreads each block's av
                # chain across the next block's 12 exp gaps so its DVE
                # normalize overlaps dripped PE work instead of stalling PE
                if urgent:
                    cost, op = urgent.popleft()
                    op()
                    budget -= max(cost, 20)
                while budget > 0:
                    cost_op = next(prep[0], None)
                    if cost_op is None:
                        cost_op = next(bulk[0], None)
                        if cost_op is None:
                            return
                    cost, op = cost_op
                    op()
                    budget -= max(cost, 20)

            def s_exp_block(b, qq, kk, budget=390):
                col0 = b * N
                e16 = []
                for h in range(HEADS):
                    c6 = h // 2
                    p0 = (h % 2) * 64
                    q_ap = qq[c6][p0:p0 + 64, col0:col0 + N]
                    sps = s_ps.tile([MC, 2 * N], F32, tag="S", name="S")
                    for j in range(2):
                        k_ap = kk[c6][p0:p0 + 64,
                                      col0 + j * MC:col0 + (j + 1) * MC]
                        nc.tensor.matmul(sps[:, j * N:(j + 1) * N],
                                         k_ap, q_ap, start=True, stop=True)
                    e = es_pool.tile([MC, 2 * N], F16, tag=f"e16_{h}",
                                     name=f"e16_{h}")
                    nc.scalar.activation(e, sps, AF.Exp, scale=ESC)
                    e16.append(e)
                    drip(budget)
                return e16

            # merged block order: first x blocks slot into y's ACT tail.
            # proj chunks enter the weave two blocks after their zT rows
            # land, shifting PE filler toward ACT-bound stretches.
            ztg_y = [(zTy8["h"].rearrange("p (c n) -> p c n", c=CH),
                      zTy8["l"].rearrange("p (c n) -> p c n", c=CH), CH, 0)]
            ztg_x = [(xin_pair(xt8, "h", kp), xin_pair(xt8, "l", kp),
                      2, kp * 2)
                     for kp in range(3)]
            cfg = {"y": (qy6, ky6, "y", "y1", "yo", ztg_y),
                   "x": (qx6, kx6, "x", "x1", "xo", ztg_x)}
            zt_views = {"y": zpy, "x": zpx}
            order = ([("y", b) for b in range(6)]
                     + [("x", 0), ("y", 6), ("x", 1), ("y", 7)]
                     + [("x", b) for b in range(2, BC)])
            lag = {"y": deque(), "x": deque()}

            def queue_proj(sd, pb, erot="vva", split_dma=False):
                qq, kk, zkey, o1, o2, _ztg = cfg[sd]
                for t in proj_sched[pb]:
                    bulk[0] = chain(bulk[0], proj_ops(
                        t, zt_views[zkey],
                        ((wp8[1], outs_d[o1]), (wp8[2], outs_d[o2])),
                        erot=erot, split_dma=split_dma))

            # x-side prep woven into the y region
            prep[0] = chain(v_ops(),
                            qkv_ops("q", xt8, qx6, "va"),
                            se_ops(),
                            weave(qkv_ops("k", xt8, kx6, "va"),
                                  se_gates(), 8))
            prev = None
            for oi, (sd, b) in enumerate(order):
                if prev is not None:
                    psd, pb, pe = prev
                    urgent.extend(av_ops(pb, pe, cfg[psd][5]))
                    lag[psd].append(pb)
                    if len(lag[psd]) >= 2:
                        queue_proj(psd, lag[psd].popleft())
                if oi >= len(order) - 3:
                    # tail: drain lagged projections early so the last
                    # blocks' PE filler doesn't pile up after the exp chain
                    for s2 in ("y", "x"):
                        while lag[s2]:
                            queue_proj(s2, lag[s2].popleft(), erot="av")
                if (sd, b) == ("x", 0):
                    # x blocks read qx/kx/SE outputs; emit any remaining
                    # prep pieces now (normally already drained)
                    for cost, op in prep[0]:
                        op()
                    prep[0] = iter(())
                bgt = 390
                e16 = s_exp_block(b, *cfg[sd][:2], budget=bgt)
                prev = (sd, b, e16)
            psd, pb, pe = prev
            urgent.extend(av_ops(pb, pe, cfg[psd][5]))
            lag[psd].append(pb)
            for sd in ("y", "x"):
                while lag[sd]:
                    queue_proj(sd, lag[sd].popleft(), erot="av",
                               split_dma=True)
            while urgent:
                urgent.popleft()[1]()
            for cost, op in bulk[0]:
                op()


def _f8(a):
    return np.clip(a, -240.0, 240.0).astype(ml_dtypes.float8_e4m3)


def _split8(a):
    hi = _f8(a)
    lo = _f8(np.asarray(a, np.float64) - hi.astype(np.float64))
    return hi, lo


def _toeplitz(k5):
    """[144,144] T with T[m,n] = k5[my-ny+2, mx-nx+2]"""
    t = np.zeros((N, N), np.float64)
    for ny in range(PATCH):
        for nx in range(PATCH):
            for dy in range(-2, 3):
                for dx in range(-2, 3):
                    my, mx = ny + dy, nx + dx
                    if 0 <= my < PATCH and 0 <= mx < PATCH:
                        t[my * PATCH + mx, ny * PATCH + nx] = k5[dy + 2, dx + 2]
    return t


def _pack8(a):
    """[768, D] fp -> [1536, D] fp8: per 256-row chunk, hi then lo"""
    hi, lo = _split8(a)
    return np.concatenate(
        [blk for c in range(3)
         for blk in (hi[c * 256:(c + 1) * 256], lo[c * 256:(c + 1) * 256])])


def _prep_weights(inputs):
    Wqkv = np.asarray(inputs["Wqkv"], np.float64)
    w = {}
    for i, nm in enumerate("qkv"):
        w[f"w{nm}p"] = _pack8(WS * Wqkv[:, (i + 1) * DIM:(i + 2) * DIM])
    wp64 = np.asarray(inputs["Wproj"], np.float64)
    w["wp1p"] = _pack8(WS * wp64)
    w["wp2p"] = _pack8(WS * (wp64 @ wp64))
    bp = np.asarray(inputs["bproj"], np.float64).reshape(1, DIM)
    bp2 = (bp @ wp64 + bp).astype(np.float32)
    sw1_64 = np.asarray(inputs["se_w1"], np.float64)
    w["sw1"] = sw1_64.astype(np.float16)
    wq64 = Wqkv[:, DIM:2 * DIM]
    w["sw1q"] = ((wq64 @ sw1_64) / N).astype(np.float16)
    wqmean = wq64.mean(axis=1)
    w["sw2"] = np.asarray(inputs["se_w2"], np.float16)
    sa_w = np.asarray(inputs["sa_w"], np.float64)
    w["tm"] = _toeplitz(sa_w[0, 0]).astype(np.float16)
    w["tx"] = _toeplitz(sa_w[0, 1]).astype(np.float16)
    w["cb"] = np.asarray(inputs["sa_b"], np.float32).reshape(1, 1)
    return w, bp.astype(np.float32), bp2, wqmean


def kernel(**inputs):
    from concourse.bass_utils import run_bass_kernel_spmd

    if "nc" not in _COMPILED:
        _COMPILED["nc"] = build_program()
    nc = _COMPILED["nc"]

    w, bp, bp2, wqmean = _prep_weights(inputs)
    x = np.asarray(inputs["x"], np.float32).reshape(B, N, DIM)
    y = np.asarray(inputs["y"], np.float32).reshape(B, N, DIM)
    in_maps = []
    for i in range(NCORES):
        m = dict(w)
        xsh = x[i * BC:(i + 1) * BC].reshape(NT, DIM)
        ysh = y[i * BC:(i + 1) * BC].reshape(NT, DIM)
        m["xp"] = _pack8(np.ascontiguousarray(xsh.T))
        m["yp"] = _pack8(np.ascontiguousarray(ysh.T))
        m["xsum"] = np.ascontiguousarray(
            WS * xsh.reshape(BC, N, DIM).sum(axis=1, dtype=np.float64).T
        ).astype(np.float16)
        m["msd"] = np.ascontiguousarray(
            WS * (ysh.astype(np.float64) @ wqmean).reshape(BC, N).T
        ).astype(np.float16)
        in_maps.append(m)

    res = run_bass_kernel_spmd(nc, in_maps, core_ids=list(range(NCORES)))
    inv = 1.0 / (WS * WS)
    outs = []
    for name, bias in (("x1", bp), ("y1", bp), ("xo", bp2), ("yo", bp2)):
        full = np.concatenate(
            [np.asarray(res.results[i][name], np.float32).reshape(BC, N, DIM)
             for i in range(NCORES)], axis=0)
        outs.append(full * inv + bias.reshape(1, 1, DIM))
    return tuple(outs)
